# revision 1
# baseline (speedup 1.0000x reference)
"""Trainium2 Bass kernel for nn_BiARMA (2-layer ARMA GNN, K=2 stacks, T=2).

Math: A = D^-1/2 C D^-1/2 (C = edge-count matrix, deg by dst).
Key identity: norm[e] = dinv[src]*dinv[dst] factors, so
  segment_sum(out[src]*norm, dst) = dinv_dst * gather_sum(dinv_src*out[src])
-> every message-passing round is a pure row-gather-accumulate of a
pre-scaled node tensor.  Weights commute with aggregation, so matmuls
run on the aggregated tensor.

Distribution: dst-nodes sharded over 8 cores (graph parallel).  Each
core keeps a full replica of the current pre-scaled node tensor in its
DRAM, gathers rows for its local edges with the gpsimd dma_gather
ucode op (int16 indices -> replica split into two <=32K-row halves,
per-block padded A/B slot ranges), reduces padded per-node slots on
DVE, applies weights on PE, and AllGathers its updated shard each
round.
"""

import sys
from dataclasses import dataclass, field

import numpy as np

sys.path.insert(0, "/opt/trn_rl_repo")

P = 128


@dataclass
class Cfg:
    N: int = 50000
    E: int = 800000
    IN_C: int = 64
    HID_C: int = 64
    OUT_C: int = 32
    K: int = 2
    CORES: int = 8
    # gather-tile budget, f32 elements per partition per group
    group_budget_elems: int = 8192

    @property
    def npc_raw(self):
        return self.N // self.CORES

    @property
    def blocks(self):
        # +1 guarantees at least one phantom row per core, so the pad
        # rows (last row of each replica half) are never real nodes
        return (self.npc_raw + 1 + P - 1) // P

    @property
    def NPC(self):
        return self.blocks * P

    @property
    def NREP(self):
        return self.CORES * self.NPC

    @property
    def HALF(self):  # rows per replica half (cores 0-3 | 4-7)
        return self.CORES // 2 * self.NPC

    @property
    def PAD_LOC(self):  # pad row, local to each half (a phantom row)
        return self.HALF - 1


@dataclass
class Struct:
    DA: list
    DB: list
    col_off: list       # per-block column offset (A+B combined)
    a_off: list         # per-block offset into the A column space
    b_off: list         # per-block offset into the B column space
    tot_cols: int
    idx16: np.ndarray   # [CORES, 128, tot_cols*8] int16 wrapped+replicated
    idx32: np.ndarray   # [CORES, P, tot_cols] int32, -1 padded (deg helper)
    pid: np.ndarray
    groups: dict = field(default_factory=dict)


def build_structure(edge_index: np.ndarray, cfg: Cfg) -> Struct:
    src = np.asarray(edge_index[0], dtype=np.int64)
    dst = np.asarray(edge_index[1], dtype=np.int64)
    N, CORES, NPC, NB = cfg.N, cfg.CORES, cfg.NPC, cfg.blocks

    deg = np.bincount(dst, minlength=N)
    order = np.argsort(-deg, kind="stable")
    core_of = np.empty(N, np.int64)
    local_of = np.empty(N, np.int64)
    core_of[order] = np.arange(N) % CORES
    local_of[order] = np.arange(N) // CORES
    pid = core_of * NPC + local_of

    ecore = core_of[dst]
    dloc = local_of[dst]
    spid = pid[src]
    half = (spid >= cfg.HALF).astype(np.int64)
    sloc = spid - half * cfg.HALF  # index within its half

    # per (core, node, half) counts -> per-block padded A/B widths
    cnt = np.zeros((2, CORES, NPC), np.int64)
    for h in (0, 1):
        for c in range(CORES):
            m = (ecore == c) & (half == h)
            cnt[h, c] = np.bincount(dloc[m], minlength=NPC)
    DA = cnt[0].reshape(CORES, NB, P).max(axis=(0, 2))
    DB = cnt[1].reshape(CORES, NB, P).max(axis=(0, 2))
    DA = np.maximum(DA, 1).tolist()
    DB = np.maximum(DB, 1).tolist()
    D = [DA[b] + DB[b] for b in range(NB)]
    col_off = np.concatenate([[0], np.cumsum(D)]).tolist()
    a_off = [col_off[b] for b in range(NB)]          # A slots first per block
    b_off = [col_off[b] + DA[b] for b in range(NB)]  # then B slots
    tot_cols = int(col_off[-1])

    # per-slot values, node-major layout [P, tot_cols]
    vals = np.full((CORES, P, tot_cols), -1, np.int64)
    eo = np.lexsort((dloc, ecore))
    ecore_s, dloc_s, sloc_s, half_s = ecore[eo], dloc[eo], sloc[eo], half[eo]
    aoff = np.asarray(a_off)
    boff = np.asarray(b_off)
    for c in range(CORES):
        m = ecore_s == c
        dl, sl, hh = dloc_s[m], sloc_s[m], half_s[m]
        for h in (0, 1):
            mh = hh == h
            dlh, slh = dl[mh], sl[mh]
            cth = np.bincount(dlh, minlength=NPC)
            starts = np.concatenate([[0], np.cumsum(cth)])[:-1]
            pos = np.arange(dlh.shape[0]) - starts[dlh]
            b = dlh // P
            p = dlh % P
            col = (aoff if h == 0 else boff)[b] + pos
            vals[c, p, col] = slh

    # int32 deg helper (-1 = pad)
    idx32 = vals.astype(np.int32)

    # int16 gather arrays, wrapped [16, .] + replicated to 128 partitions.
    # Call order per round: groups of consecutive blocks; per group, call A
    # covers the blocks' A-columns (in block order), call B the B-columns.
    # Flat index i (within a call) -> (partition i%128, out column i//128);
    # wrapped storage (i%16, i//16).
    v16 = np.where(vals < 0, cfg.PAD_LOC, vals).astype(np.int16)
    # For each global column c (0..tot_cols) and partition p the flat call
    # position is determined per call; we store per-column wrapped chunks so
    # slicing per call is contiguous: column-major chunks of 8 wrapped cols.
    # Build the per-call concatenated layout at group-construction time
    # below instead, once groups are known.

    st = Struct(DA=DA, DB=DB, col_off=col_off, a_off=a_off, b_off=b_off,
                tot_cols=tot_cols, idx16=None, idx32=idx32, pid=pid)

    def make_groups(F):
        budget = max(cfg.group_budget_elems // F, max(D))
        groups = []
        b0 = 0
        while b0 < NB:
            b1 = b0
            tot = 0
            while b1 < NB and (tot + D[b1] <= budget or b1 == b0):
                tot += D[b1]
                b1 += 1
            groups.append((b0, b1))
            b0 = b1
        return groups

    KIN1 = cfg.K * cfg.HID_C
    st.groups = {
        1: make_groups(cfg.IN_C),
        2: make_groups(KIN1),
        3: make_groups(cfg.HID_C),
        4: make_groups(cfg.K * cfg.OUT_C),
    }

    # The idx16 storage layout is independent of grouping: for ANY contiguous
    # block range, call A reads columns [a_slots of b0..b1) in block order and
    # call B reads [b_slots of b0..b1).  To keep every call's index slice
    # contiguous in SBUF we store TWO wrapped arrays: one concatenating all
    # A-columns (block-major), one all B-columns, then concatenate them.
    # Per-block offsets within those spaces:
    a_cum = np.concatenate([[0], np.cumsum(DA)]).astype(np.int64)
    b_cum = np.concatenate([[0], np.cumsum(DB)]).astype(np.int64)
    TA, TB = int(a_cum[-1]), int(b_cum[-1])
    st.a_cum = a_cum.tolist()
    st.b_cum = b_cum.tolist()
    st.TA, st.TB = TA, TB

    idx16 = np.empty((CORES, 16, (TA + TB) * 8), np.int16)
    for c in range(CORES):
        # gather values into call-A column space [P, TA] and call-B [P, TB]
        va = np.empty((P, TA), np.int16)
        vb = np.empty((P, TB), np.int16)
        for b in range(NB):
            va[:, a_cum[b]:a_cum[b + 1]] = \
                v16[c][:, a_off[b]:a_off[b] + DA[b]]
            vb[:, b_cum[b]:b_cum[b + 1]] = \
                v16[c][:, b_off[b]:b_off[b] + DB[b]]
        both = np.concatenate([va, vb], axis=1)  # [P, TA+TB]
        # column c', partition p -> flat i = c'*128 + p -> (i%16, i//16):
        # wrapped[r, c'*8 + q] with p = q*16 + r
        w = both.reshape(16, 8, TA + TB, order="F")  # p=(q,r): r fastest
        # both[p, c'] with p = q*16+r -> want w2[r, c', q]
        w2 = np.transpose(w, (0, 2, 1)).reshape(16, (TA + TB) * 8)
        idx16[c] = w2
    st.idx16 = np.tile(idx16, (1, 8, 1))  # replicate per Q7 core -> [., 128, .]
    return st


def build_weight_inputs(inp: dict, cfg: Cfg) -> dict:
    K, IN_C, HID_C, OUT_C = cfg.K, cfg.IN_C, cfg.HID_C, cfg.OUT_C
    f4 = lambda a: np.ascontiguousarray(a, dtype=np.float32)

    rootw1 = np.transpose(inp["root_w1"][0], (1, 0, 2)).reshape(IN_C, K * HID_C)
    b1row = inp["b1"][0, :, 0, :].reshape(1, K * HID_C)
    initw1 = np.transpose(inp["init_w1"], (1, 0, 2)).reshape(IN_C, K * HID_C)
    w1bd = np.zeros((K * HID_C, K * HID_C), np.float32)
    for k in range(K):
        w1bd[k * HID_C:(k + 1) * HID_C, k * HID_C:(k + 1) * HID_C] = inp["w1"][0, k]

    # 0.5 absorbed: round-2 h-stage feeds the UNhalved stack sum into root2
    rootw2 = 0.5 * np.transpose(inp["root_w2"][0], (1, 0, 2)).reshape(HID_C, K * OUT_C)
    b2row = inp["b2"][0, :, 0, :].reshape(1, K * OUT_C)
    initw2 = np.transpose(inp["init_w2"], (1, 0, 2)).reshape(HID_C, K * OUT_C)
    w2bd = np.zeros((K * OUT_C, K * OUT_C), np.float32)
    for k in range(K):
        w2bd[k * OUT_C:(k + 1) * OUT_C, k * OUT_C:(k + 1) * OUT_C] = inp["w2"][0, k]

    return {
        "w_rootw1": f4(rootw1), "w_b1": f4(b1row), "w_initw1": f4(initw1),
        "w_w1bd": f4(w1bd), "w_rootw2": f4(rootw2), "w_b2": f4(b2row),
        "w_initw2": f4(initw2), "w_w2bd": f4(w2bd),
    }


def build_nc(cfg: Cfg, st: Struct):
    import concourse.bacc as bacc
    import concourse.bass as bass
    import concourse.mybir as mybir
    import concourse.tile as tile
    from concourse import library_config
    from concourse.masks import make_identity

    f32 = mybir.dt.float32
    i16 = mybir.dt.int16
    i32 = mybir.dt.int32
    X = mybir.AxisListType.X
    Alu = mybir.AluOpType
    Act = mybir.ActivationFunctionType

    K, IN_C, HID_C, OUT_C = cfg.K, cfg.IN_C, cfg.HID_C, cfg.OUT_C
    G1 = K * HID_C   # 128
    G2 = K * OUT_C   # 64
    NB = cfg.blocks
    NPC, NREP, HALF = cfg.NPC, cfg.NREP, cfg.HALF
    DA, DB = st.DA, st.DB
    TA, TB = st.TA, st.TB
    a_cum, b_cum = st.a_cum, st.b_cum
    WTOT = (TA + TB) * 8

    nc = bacc.Bacc(
        "TRN2",
        target_bir_lowering=False,
        debug=False,
        num_devices=cfg.CORES,
    )

    # ---- kernel I/O ----
    xs = nc.dram_tensor("xs", [NPC, IN_C], f32, kind="ExternalInput")
    idx16_d = nc.dram_tensor("idx16", [P, WTOT], i16, kind="ExternalInput")
    idx32_d = nc.dram_tensor("idx32", [P, st.tot_cols], i32, kind="ExternalInput")
    w_rootw1 = nc.dram_tensor("w_rootw1", [IN_C, G1], f32, kind="ExternalInput")
    w_b1 = nc.dram_tensor("w_b1", [1, G1], f32, kind="ExternalInput")
    w_initw1 = nc.dram_tensor("w_initw1", [IN_C, G1], f32, kind="ExternalInput")
    w_w1bd = nc.dram_tensor("w_w1bd", [G1, G1], f32, kind="ExternalInput")
    w_rootw2 = nc.dram_tensor("w_rootw2", [HID_C, G2], f32, kind="ExternalInput")
    w_b2 = nc.dram_tensor("w_b2", [1, G2], f32, kind="ExternalInput")
    w_initw2 = nc.dram_tensor("w_initw2", [HID_C, G2], f32, kind="ExternalInput")
    w_w2bd = nc.dram_tensor("w_w2bd", [G2, G2], f32, kind="ExternalInput")
    out_d = nc.dram_tensor("out", [NPC, OUT_C], f32, kind="ExternalOutput")

    # ---- internal DRAM ----
    y = {
        1: nc.dram_tensor("y1", [NREP, IN_C], f32, addr_space="Shared"),
        2: nc.dram_tensor("y2", [NREP, G1], f32, addr_space="Shared"),
        3: nc.dram_tensor("y3", [NREP, HID_C], f32, addr_space="Shared"),
        4: nc.dram_tensor("y4", [NREP, G2], f32, addr_space="Shared"),
    }
    ag_in = {
        1: nc.dram_tensor("agin1", [NPC, IN_C], f32),
        2: nc.dram_tensor("agin2", [NPC, G1], f32),
        3: nc.dram_tensor("agin3", [NPC, HID_C], f32),
        4: nc.dram_tensor("agin4", [NPC, G2], f32),
    }
    FW = {1: IN_C, 2: G1, 3: HID_C, 4: G2}
    GW = {1: G1, 2: G1, 3: G2, 4: G2}

    rg = [list(range(cfg.CORES))]

    max_gt_elems = max(
        max((st.col_off[b1] - st.col_off[b0]) * FW[r]
            for (b0, b1) in st.groups[r])
        for r in (1, 2, 3, 4)
    )

    dsem = nc.alloc_semaphore("gsem")
    n_dma = [0]  # cumulative dma_gather count (crit mode)
    NSEM = 8
    dsems = [nc.alloc_semaphore(f"gsem{i}") for i in range(NSEM)]
    sem_count = [0] * NSEM
    gidx = [0]  # global group counter
    wait_tgt = [None]  # (sem, value) the current group's consumers wait on

    with tile.TileContext(nc) as tc:
        with (
            tc.tile_pool(name="const", bufs=1) as cpool,
            tc.tile_pool(name="gather", bufs=3) as gpool,
            tc.tile_pool(name="work", bufs=3) as wpool,
            tc.tile_pool(name="psum", bufs=3, space="PSUM") as ppool,
        ):
            # ---------- constants ----------
            ident = cpool.tile([P, P], f32)
            make_identity(nc, ident[:])

            def load_w(t, shape, tag):
                s = cpool.tile(list(shape), f32, tag=tag)
                nc.sync.dma_start(out=s[:], in_=t[:, :])
                return s

            rootw1_s = load_w(w_rootw1, (IN_C, G1), "w_rootw1")
            initw1_s = load_w(w_initw1, (IN_C, G1), "w_initw1")
            w1bd_s = load_w(w_w1bd, (G1, G1), "w_w1bd")
            rootw2_s = load_w(w_rootw2, (HID_C, G2), "w_rootw2")
            initw2_s = load_w(w_initw2, (HID_C, G2), "w_initw2")
            w2bd_s = load_w(w_w2bd, (G2, G2), "w_w2bd")
            b1_s = load_w(w_b1, (1, G1), "w_b1")
            b2_s = load_w(w_b2, (1, G2), "w_b2")
            rhs_s = {1: initw1_s, 2: w1bd_s, 3: initw2_s, 4: w2bd_s}

            ones1 = cpool.tile([1, P], f32)
            nc.vector.memset(ones1[:], 1.0)
            b1rep = cpool.tile([P, G1], f32)
            b2rep = cpool.tile([P, G2], f32)
            bps = ppool.tile([P, G1], f32, tag="mmps")
            nc.tensor.matmul(bps[:], lhsT=ones1[:], rhs=b1_s[:], start=True, stop=True)
            nc.vector.tensor_copy(b1rep[:], bps[:])
            bps2 = ppool.tile([P, G2], f32, tag="mmps")
            nc.tensor.matmul(bps2[:], lhsT=ones1[:], rhs=b2_s[:], start=True, stop=True)
            nc.vector.tensor_copy(b2rep[:], bps2[:])

            # ---------- gather indices ----------
            idx16_s = cpool.tile([P, WTOT], i16)
            nc.sync.dma_start(out=idx16_s[:], in_=idx16_d[:, :])

            # ---------- degrees (from the -1-padded int32 helper) ----------
            root1 = cpool.tile([P, NB, G1], f32)
            root2 = cpool.tile([P, NB, G2], f32)
            dinv = cpool.tile([P, NB], f32)
            dinvh = cpool.tile([P, NB], f32)
            with tc.tile_pool(name="prolog", bufs=1) as qpool:
                idx32_s = qpool.tile([P, st.tot_cols], i32)
                nc.sync.dma_start(out=idx32_s[:], in_=idx32_d[:, :])
                idxf = qpool.tile([P, st.tot_cols], f32)
                nc.vector.tensor_copy(idxf[:], idx32_s[:])
                valid = qpool.tile([P, st.tot_cols], f32)
                nc.vector.tensor_single_scalar(
                    valid[:], idxf[:], -1.0, Alu.not_equal
                )
                deg = qpool.tile([P, NB], f32)
                for b in range(NB):
                    c0, c1 = st.col_off[b], st.col_off[b + 1]
                    nc.vector.reduce_sum(deg[:, b:b + 1], valid[:, c0:c1], axis=X)
                degc = qpool.tile([P, NB], f32)
                nc.vector.tensor_scalar_max(degc[:], deg[:], 1.0)
                sq = qpool.tile([P, NB], f32)
                nc.scalar.activation(sq[:], degc[:], Act.Sqrt)
                rinv = qpool.tile([P, NB], f32)
                nc.vector.reciprocal(rinv[:], sq[:])
                mask = qpool.tile([P, NB], f32)
                nc.vector.tensor_single_scalar(mask[:], deg[:], 0.0, Alu.is_gt)
                nc.vector.tensor_mul(dinv[:], rinv[:], mask[:])
                nc.vector.tensor_scalar_mul(dinvh[:], dinv[:], 0.5)

                # ---------- roots + Y1 ----------
                x_s = qpool.tile([P, NB, IN_C], f32)
                for b in range(NB):
                    nc.sync.dma_start(
                        out=x_s[:, b, :], in_=xs[b * P:(b + 1) * P, :]
                    )
                for b in range(NB):
                    dcol = dinv[:, b:b + 1]
                    xT_ps = ppool.tile([IN_C, P], f32, tag="tps")
                    nc.tensor.transpose(xT_ps[:], x_s[:, b, :], ident[:])
                    xT = wpool.tile([IN_C, P], f32, tag="aggT")
                    nc.scalar.activation(xT[:], xT_ps[:], Act.Copy)
                    r1_ps = ppool.tile([P, G1], f32, tag="mmps")
                    nc.tensor.matmul(
                        r1_ps[:], lhsT=xT[:], rhs=rootw1_s[:], start=True, stop=True
                    )
                    nc.vector.tensor_add(root1[:, b, :], r1_ps[:], b1rep[:])
                    y1b = wpool.tile([P, IN_C], f32, tag="yout")
                    nc.scalar.activation(y1b[:], x_s[:, b, :], Act.Copy, scale=dcol)
                    nc.sync.dma_start(
                        out=ag_in[1][b * P:(b + 1) * P, :], in_=y1b[:]
                    )
            nc.gpsimd.collective_compute(
                "AllGather", Alu.bypass, replica_groups=rg,
                ins=[ag_in[1].ap().opt()], outs=[y[1].ap().opt()],
            )

            # ---------- 4 message-passing rounds ----------
            import os
            max_round = int(os.environ.get("GNN_STAGE", "4"))
            reps = int(os.environ.get("GNN_REPS", "1"))

            def gather_group(r, b0, b1):
                """Issue the A and B dma_gather calls for blocks [b0,b1);
                returns the gather tile (cols: A of b0..b1, then B)."""
                F = FW[r]
                nA = (a_cum[b1] - a_cum[b0]) * P
                nB = (b_cum[b1] - b_cum[b0]) * P
                if os.environ.get("GNN_TINY_GATHER"):
                    # timing experiment: same call count, 128 idx each
                    gt = gpool.tile([P, max_gt_elems], f32, tag="gt")
                    s = gidx[0] % NSEM
                    gidx[0] += 1
                    sem_count[s] += 32
                    nc.gpsimd.dma_gather(
                        out_ap=gt[:, :F].rearrange("p (c f) -> p c f", f=F),
                        in_ap=y[r].ap(), idxs_ap=idx16_s[:, :8],
                        num_idxs=P, num_idxs_reg=P, elem_size=F,
                        single_packet=False,
                    ).then_inc(dsems[s], 16)
                    nc.gpsimd.dma_gather(
                        out_ap=gt[:, F:2 * F].rearrange("p (c f) -> p c f", f=F),
                        in_ap=y[r].ap(), idxs_ap=idx16_s[:, :8],
                        num_idxs=P, num_idxs_reg=P, elem_size=F,
                        single_packet=False,
                    ).then_inc(dsems[s], 16)
                    wait_tgt[0] = (dsems[s], sem_count[s])
                    return gt, (a_cum[b1] - a_cum[b0])
                ncols = (nA + nB) // P
                gt = gpool.tile([P, max_gt_elems], f32, tag="gt")
                outA = gt[:, :nA // P * F].rearrange("p (c f) -> p c f", f=F)
                outB = gt[:, nA // P * F:ncols * F].rearrange(
                    "p (c f) -> p c f", f=F
                )
                ixA = idx16_s[:, a_cum[b0] * 8:a_cum[b1] * 8]
                ixB = idx16_s[:, (TA + b_cum[b0]) * 8:(TA + b_cum[b1]) * 8]
                yv = y[r]
                if os.environ.get("GNN_SYNC_CRIT"):
                    with tc.tile_critical():
                        nc.gpsimd.dma_gather(
                            out_ap=outA, in_ap=yv.ap(), idxs_ap=ixA,
                            num_idxs=nA, num_idxs_reg=nA, elem_size=F,
                            single_packet=False,
                        ).then_inc(dsem, 16)
                        nc.gpsimd.dma_gather(
                            out_ap=outB,
                            in_ap=yv[HALF:NREP, :],
                            idxs_ap=ixB,
                            num_idxs=nB, num_idxs_reg=nB, elem_size=F,
                            single_packet=False,
                        ).then_inc(dsem, 16)
                        n_dma[0] += 2
                        nc.gpsimd.wait_ge(dsem, 16 * n_dma[0])
                else:
                    s = gidx[0] % NSEM
                    gidx[0] += 1
                    sem_count[s] += 32
                    nc.gpsimd.dma_gather(
                        out_ap=outA, in_ap=yv.ap(), idxs_ap=ixA,
                        num_idxs=nA, num_idxs_reg=nA, elem_size=F,
                        single_packet=False,
                    ).then_inc(dsems[s], 16)
                    nc.gpsimd.dma_gather(
                        out_ap=outB,
                        in_ap=yv[HALF:NREP, :],
                        idxs_ap=ixB,
                        num_idxs=nB, num_idxs_reg=nB, elem_size=F,
                        single_packet=False,
                    ).then_inc(dsems[s], 16)
                    wait_tgt[0] = (dsems[s], sem_count[s])
                return gt, nA // P

            for rep in range(reps):
              for r in (1, 2, 3, 4):
                if r > max_round:
                    break
                F = FW[r]
                G = GW[r]
                for (b0, b1) in st.groups[r]:
                    gt, colsA = gather_group(r, b0, b1)
                    if os.environ.get("GNN_R1_LITE"):
                        lite_t = wpool.tile([P, FW[r]], f32, tag="agg")
                        nc.vector.tensor_copy(lite_t[:], gt[:, :FW[r]])
                        continue
                    for b in range(b0, b1):
                        dcol = dinv[:, b:b + 1]
                        # A slots at cols [a_cum[b]-a_cum[b0], +DA[b]),
                        # B slots at colsA + [b_cum[b]-b_cum[b0], +DB[b])
                        oA = a_cum[b] - a_cum[b0]
                        oB = colsA + (b_cum[b] - b_cum[b0])
                        aggA = wpool.tile([P, F], f32, tag="aggA")
                        rA = nc.vector.reduce_sum(
                            aggA[:],
                            gt[:, oA * F:(oA + DA[b]) * F].rearrange(
                                "p (d f) -> p f d", f=F
                            ),
                            axis=X,
                        )
                        aggB = wpool.tile([P, F], f32, tag="aggB")
                        rB = nc.vector.reduce_sum(
                            aggB[:],
                            gt[:, oB * F:(oB + DB[b]) * F].rearrange(
                                "p (d f) -> p f d", f=F
                            ),
                            axis=X,
                        )
                        if not os.environ.get("GNN_SYNC_CRIT"):
                            ws, wv = wait_tgt[0]
                            rA._wait_ge(ws, wv)
                            rB._wait_ge(ws, wv)
                        agg = wpool.tile([P, F], f32, tag="agg")
                        nc.vector.tensor_add(agg[:], aggA[:], aggB[:])
                        aggT_ps = ppool.tile([F, P], f32, tag="tps")
                        nc.tensor.transpose(aggT_ps[:], agg[:], ident[:])
                        aggT = wpool.tile([F, P], f32, tag="aggT")
                        nc.scalar.activation(aggT[:], aggT_ps[:], Act.Copy)
                        mm_ps = ppool.tile([P, G], f32, tag="mmps")
                        nc.tensor.matmul(
                            mm_ps[:], lhsT=aggT[:], rhs=rhs_s[r][:],
                            start=True, stop=True,
                        )
                        root = root1 if r <= 2 else root2
                        t_sb = wpool.tile([P, G], f32, tag="tsb")
                        nc.vector.scalar_tensor_tensor(
                            t_sb[:], mm_ps[:], dcol, root[:, b, :],
                            op0=Alu.mult, op1=Alu.add,
                        )
                        if r == 1:
                            yo = wpool.tile([P, G1], f32, tag="yout")
                            nc.scalar.activation(yo[:], t_sb[:], Act.Relu, scale=dcol)
                            nc.sync.dma_start(
                                out=ag_in[2][b * P:(b + 1) * P, :], in_=yo[:]
                            )
                        elif r == 2:
                            out1 = wpool.tile([P, G1], f32, tag="out1")
                            nc.scalar.activation(out1[:], t_sb[:], Act.Relu)
                            hsum = wpool.tile([P, HID_C], f32, tag="hsum")
                            nc.vector.tensor_add(
                                hsum[:], out1[:, :HID_C], out1[:, HID_C:]
                            )
                            yo = wpool.tile([P, HID_C], f32, tag="yout")
                            nc.scalar.activation(
                                yo[:], hsum[:], Act.Copy, scale=dinvh[:, b:b + 1]
                            )
                            nc.sync.dma_start(
                                out=ag_in[3][b * P:(b + 1) * P, :], in_=yo[:]
                            )
                            hT_ps = ppool.tile([HID_C, P], f32, tag="tps")
                            nc.tensor.transpose(hT_ps[:], hsum[:], ident[:])
                            hT = wpool.tile([HID_C, P], f32, tag="aggT")
                            nc.scalar.activation(hT[:], hT_ps[:], Act.Copy)
                            r2_ps = ppool.tile([P, G2], f32, tag="mmps")
                            nc.tensor.matmul(
                                r2_ps[:], lhsT=hT[:], rhs=rootw2_s[:],
                                start=True, stop=True,
                            )
                            nc.vector.tensor_add(root2[:, b, :], r2_ps[:], b2rep[:])
                        elif r == 3:
                            yo = wpool.tile([P, G2], f32, tag="yout")
                            nc.scalar.activation(yo[:], t_sb[:], Act.Relu, scale=dcol)
                            nc.sync.dma_start(
                                out=ag_in[4][b * P:(b + 1) * P, :], in_=yo[:]
                            )
                        else:
                            ofin = wpool.tile([P, G2], f32, tag="out1")
                            nc.scalar.activation(ofin[:], t_sb[:], Act.Relu)
                            msum = wpool.tile([P, OUT_C], f32, tag="hsum")
                            nc.vector.tensor_add(
                                msum[:], ofin[:, :OUT_C], ofin[:, OUT_C:]
                            )
                            yo = wpool.tile([P, OUT_C], f32, tag="yout")
                            nc.scalar.activation(yo[:], msum[:], Act.Copy, scale=0.5)
                            nc.sync.dma_start(
                                out=out_d[b * P:(b + 1) * P, :], in_=yo[:]
                            )
                if r < 4 and r < max_round and not os.environ.get("GNN_SKIP_AG"):
                    nc.gpsimd.collective_compute(
                        "AllGather", Alu.bypass, replica_groups=rg,
                        ins=[ag_in[r + 1].ap().opt()], outs=[y[r + 1].ap().opt()],
                    )

    nc.compile()
    return nc


def build_in_maps(inputs: dict, cfg: Cfg, st: Struct) -> list:
    x = np.asarray(inputs["x"], dtype=np.float32)
    wmap = build_weight_inputs(inputs, cfg)
    in_maps = []
    for c in range(cfg.CORES):
        xs = np.zeros((cfg.NPC, cfg.IN_C), np.float32)
        mine = np.nonzero(st.pid // cfg.NPC == c)[0]
        loc = st.pid[mine] % cfg.NPC
        xs[loc] = x[mine]
        m = {
            "xs": xs,
            "idx16": np.ascontiguousarray(st.idx16[c]),
            "idx32": np.ascontiguousarray(st.idx32[c]),
        }
        m.update(wmap)
        in_maps.append(m)
    return in_maps


def assemble_output(results: list, cfg: Cfg, st: Struct) -> np.ndarray:
    full = np.concatenate(
        [np.asarray(results[c]["out"]) for c in range(cfg.CORES)], axis=0
    )
    return np.ascontiguousarray(full[st.pid]).astype(np.float32)


def kernel(**inputs) -> np.ndarray:
    from concourse.bass_utils import run_bass_kernel_spmd

    cfg = Cfg()
    st = build_structure(np.asarray(inputs["edge_index"]), cfg)
    nc = build_nc(cfg, st)
    in_maps = build_in_maps(inputs, cfg, st)
    res = run_bass_kernel_spmd(nc, in_maps, core_ids=list(range(cfg.CORES)))
    return assemble_output(res.results, cfg, st)


if __name__ == "__main__":
    pass



# revision 2
# speedup vs baseline: 3.2060x; 3.2060x over previous
"""Trainium2 Bass kernel for nn_BiARMA (2-layer ARMA GNN, K=2 stacks, T=2).

Math: A = D^-1/2 C D^-1/2 (C = edge-count matrix, deg by dst).
Key identity: norm[e] = dinv[src]*dinv[dst] factors, so
  segment_sum(out[src]*norm, dst) = dinv_dst * gather_sum(dinv_src*out[src])
-> every message-passing round is a pure row-gather-accumulate of a
pre-scaled node tensor.  Weights commute with aggregation, so matmuls
run on the aggregated tensor.

Distribution: dst-nodes sharded over 8 cores (graph parallel).  Each
core keeps a full replica of the current pre-scaled node tensor in its
DRAM, gathers rows for its local edges with the gpsimd dma_gather
ucode op (int16 indices -> replica split into two <=32K-row halves,
per-block padded A/B slot ranges), reduces padded per-node slots on
DVE, applies weights on PE, and AllGathers its updated shard each
round.
"""

import sys
from dataclasses import dataclass, field

import numpy as np

sys.path.insert(0, "/opt/trn_rl_repo")

P = 128


@dataclass
class Cfg:
    N: int = 50000
    E: int = 800000
    IN_C: int = 64
    HID_C: int = 64
    OUT_C: int = 32
    K: int = 2
    CORES: int = 8
    # gather-tile budget, f32 elements per partition per group
    group_budget_elems: int = 8192

    @property
    def npc_raw(self):
        return self.N // self.CORES

    @property
    def blocks(self):
        # +1 guarantees at least one phantom row per core, so the pad
        # rows (last row of each replica half) are never real nodes
        return (self.npc_raw + 1 + P - 1) // P

    @property
    def NPC(self):
        return self.blocks * P

    @property
    def NREP(self):
        return self.CORES * self.NPC

    @property
    def HALF(self):  # rows per replica half (cores 0-3 | 4-7)
        return self.CORES // 2 * self.NPC

    @property
    def PAD_LOC(self):  # pad row, local to each half (a phantom row)
        return self.HALF - 1


@dataclass
class Struct:
    DA: list
    DB: list
    col_off: list       # per-block column offset (A+B combined)
    a_off: list         # per-block offset into the A column space
    b_off: list         # per-block offset into the B column space
    tot_cols: int
    idx16: np.ndarray   # [CORES, 128, tot_cols*8] int16 wrapped+replicated
    idx32: np.ndarray   # [CORES, P, tot_cols] int32, -1 padded (deg helper)
    pid: np.ndarray
    groups: dict = field(default_factory=dict)


def build_structure(edge_index: np.ndarray, cfg: Cfg) -> Struct:
    src = np.asarray(edge_index[0], dtype=np.int64)
    dst = np.asarray(edge_index[1], dtype=np.int64)
    N, CORES, NPC, NB = cfg.N, cfg.CORES, cfg.NPC, cfg.blocks

    deg = np.bincount(dst, minlength=N)
    order = np.argsort(-deg, kind="stable")
    core_of = np.empty(N, np.int64)
    local_of = np.empty(N, np.int64)
    core_of[order] = np.arange(N) % CORES
    local_of[order] = np.arange(N) // CORES
    pid = core_of * NPC + local_of

    ecore = core_of[dst]
    dloc = local_of[dst]
    spid = pid[src]
    half = (spid >= cfg.HALF).astype(np.int64)
    sloc = spid - half * cfg.HALF  # index within its half

    # per (core, node, half) counts -> per-block padded A/B widths
    cnt = np.zeros((2, CORES, NPC), np.int64)
    for h in (0, 1):
        for c in range(CORES):
            m = (ecore == c) & (half == h)
            cnt[h, c] = np.bincount(dloc[m], minlength=NPC)
    DA = cnt[0].reshape(CORES, NB, P).max(axis=(0, 2))
    DB = cnt[1].reshape(CORES, NB, P).max(axis=(0, 2))
    DA = np.maximum(DA, 1).tolist()
    DB = np.maximum(DB, 1).tolist()
    D = [DA[b] + DB[b] for b in range(NB)]
    col_off = np.concatenate([[0], np.cumsum(D)]).tolist()
    a_off = [col_off[b] for b in range(NB)]          # A slots first per block
    b_off = [col_off[b] + DA[b] for b in range(NB)]  # then B slots
    tot_cols = int(col_off[-1])

    # per-slot values, node-major layout [P, tot_cols]
    vals = np.full((CORES, P, tot_cols), -1, np.int64)
    eo = np.lexsort((dloc, ecore))
    ecore_s, dloc_s, sloc_s, half_s = ecore[eo], dloc[eo], sloc[eo], half[eo]
    aoff = np.asarray(a_off)
    boff = np.asarray(b_off)
    for c in range(CORES):
        m = ecore_s == c
        dl, sl, hh = dloc_s[m], sloc_s[m], half_s[m]
        for h in (0, 1):
            mh = hh == h
            dlh, slh = dl[mh], sl[mh]
            cth = np.bincount(dlh, minlength=NPC)
            starts = np.concatenate([[0], np.cumsum(cth)])[:-1]
            pos = np.arange(dlh.shape[0]) - starts[dlh]
            b = dlh // P
            p = dlh % P
            col = (aoff if h == 0 else boff)[b] + pos
            vals[c, p, col] = slh

    # int32 deg helper (-1 = pad)
    idx32 = vals.astype(np.int32)

    # int16 gather arrays, wrapped [16, .] + replicated to 128 partitions.
    # Call order per round: groups of consecutive blocks; per group, call A
    # covers the blocks' A-columns (in block order), call B the B-columns.
    # Flat index i (within a call) -> (partition i%128, out column i//128);
    # wrapped storage (i%16, i//16).
    v16 = np.where(vals < 0, cfg.PAD_LOC, vals).astype(np.int16)
    # For each global column c (0..tot_cols) and partition p the flat call
    # position is determined per call; we store per-column wrapped chunks so
    # slicing per call is contiguous: column-major chunks of 8 wrapped cols.
    # Build the per-call concatenated layout at group-construction time
    # below instead, once groups are known.

    st = Struct(DA=DA, DB=DB, col_off=col_off, a_off=a_off, b_off=b_off,
                tot_cols=tot_cols, idx16=None, idx32=idx32, pid=pid)

    def make_groups(F):
        budget = max(cfg.group_budget_elems // F, max(D))
        groups = []
        b0 = 0
        while b0 < NB:
            b1 = b0
            tot = 0
            while b1 < NB and (tot + D[b1] <= budget or b1 == b0):
                tot += D[b1]
                b1 += 1
            groups.append((b0, b1))
            b0 = b1
        return groups

    KIN1 = cfg.K * cfg.HID_C
    st.groups = {
        1: make_groups(cfg.IN_C),
        2: make_groups(KIN1),
        3: make_groups(cfg.HID_C),
        4: make_groups(cfg.K * cfg.OUT_C),
    }

    # The idx16 storage layout is independent of grouping: for ANY contiguous
    # block range, call A reads columns [a_slots of b0..b1) in block order and
    # call B reads [b_slots of b0..b1).  To keep every call's index slice
    # contiguous in SBUF we store TWO wrapped arrays: one concatenating all
    # A-columns (block-major), one all B-columns, then concatenate them.
    # Per-block offsets within those spaces:
    a_cum = np.concatenate([[0], np.cumsum(DA)]).astype(np.int64)
    b_cum = np.concatenate([[0], np.cumsum(DB)]).astype(np.int64)
    TA, TB = int(a_cum[-1]), int(b_cum[-1])
    st.a_cum = a_cum.tolist()
    st.b_cum = b_cum.tolist()
    st.TA, st.TB = TA, TB

    idx16 = np.empty((CORES, 16, (TA + TB) * 8), np.int16)
    for c in range(CORES):
        # gather values into call-A column space [P, TA] and call-B [P, TB]
        va = np.empty((P, TA), np.int16)
        vb = np.empty((P, TB), np.int16)
        for b in range(NB):
            va[:, a_cum[b]:a_cum[b + 1]] = \
                v16[c][:, a_off[b]:a_off[b] + DA[b]]
            vb[:, b_cum[b]:b_cum[b + 1]] = \
                v16[c][:, b_off[b]:b_off[b] + DB[b]]
        both = np.concatenate([va, vb], axis=1)  # [P, TA+TB]
        # column c', partition p -> flat i = c'*128 + p -> (i%16, i//16):
        # wrapped[r, c'*8 + q] with p = q*16 + r
        w = both.reshape(16, 8, TA + TB, order="F")  # p=(q,r): r fastest
        # both[p, c'] with p = q*16+r -> want w2[r, c', q]
        w2 = np.transpose(w, (0, 2, 1)).reshape(16, (TA + TB) * 8)
        idx16[c] = w2
    st.idx16 = np.tile(idx16, (1, 8, 1))  # replicate per Q7 core -> [., 128, .]
    return st


def build_weight_inputs(inp: dict, cfg: Cfg) -> dict:
    K, IN_C, HID_C, OUT_C = cfg.K, cfg.IN_C, cfg.HID_C, cfg.OUT_C
    f4 = lambda a: np.ascontiguousarray(a, dtype=np.float32)

    rootw1 = np.transpose(inp["root_w1"][0], (1, 0, 2)).reshape(IN_C, K * HID_C)
    b1row = inp["b1"][0, :, 0, :].reshape(1, K * HID_C)
    initw1 = np.transpose(inp["init_w1"], (1, 0, 2)).reshape(IN_C, K * HID_C)
    w1bd = np.zeros((K * HID_C, K * HID_C), np.float32)
    for k in range(K):
        w1bd[k * HID_C:(k + 1) * HID_C, k * HID_C:(k + 1) * HID_C] = inp["w1"][0, k]

    # 0.5 absorbed: round-2 h-stage feeds the UNhalved stack sum into root2
    rootw2 = 0.5 * np.transpose(inp["root_w2"][0], (1, 0, 2)).reshape(HID_C, K * OUT_C)
    b2row = inp["b2"][0, :, 0, :].reshape(1, K * OUT_C)
    initw2 = np.transpose(inp["init_w2"], (1, 0, 2)).reshape(HID_C, K * OUT_C)
    w2bd = np.zeros((K * OUT_C, K * OUT_C), np.float32)
    for k in range(K):
        w2bd[k * OUT_C:(k + 1) * OUT_C, k * OUT_C:(k + 1) * OUT_C] = inp["w2"][0, k]

    return {
        "w_rootw1": f4(rootw1), "w_b1": f4(b1row), "w_initw1": f4(initw1),
        "w_w1bd": f4(w1bd), "w_rootw2": f4(rootw2), "w_b2": f4(b2row),
        "w_initw2": f4(initw2), "w_w2bd": f4(w2bd),
    }


def build_nc(cfg: Cfg, st: Struct):
    import concourse.bacc as bacc
    import concourse.bass as bass
    import concourse.mybir as mybir
    import concourse.tile as tile
    from concourse import library_config
    from concourse.masks import make_identity

    f32 = mybir.dt.float32
    i16 = mybir.dt.int16
    i32 = mybir.dt.int32
    X = mybir.AxisListType.X
    Alu = mybir.AluOpType
    Act = mybir.ActivationFunctionType

    K, IN_C, HID_C, OUT_C = cfg.K, cfg.IN_C, cfg.HID_C, cfg.OUT_C
    G1 = K * HID_C   # 128
    G2 = K * OUT_C   # 64
    NB = cfg.blocks
    NPC, NREP, HALF = cfg.NPC, cfg.NREP, cfg.HALF
    DA, DB = st.DA, st.DB
    TA, TB = st.TA, st.TB
    a_cum, b_cum = st.a_cum, st.b_cum
    WTOT = (TA + TB) * 8

    nc = bacc.Bacc(
        "TRN2",
        target_bir_lowering=False,
        debug=False,
        num_devices=cfg.CORES,
    )

    # ---- kernel I/O ----
    xs = nc.dram_tensor("xs", [NPC, IN_C], f32, kind="ExternalInput")
    idx16_d = nc.dram_tensor("idx16", [P, WTOT], i16, kind="ExternalInput")
    idx32_d = nc.dram_tensor("idx32", [P, st.tot_cols], i32, kind="ExternalInput")
    w_rootw1 = nc.dram_tensor("w_rootw1", [IN_C, G1], f32, kind="ExternalInput")
    w_b1 = nc.dram_tensor("w_b1", [1, G1], f32, kind="ExternalInput")
    w_initw1 = nc.dram_tensor("w_initw1", [IN_C, G1], f32, kind="ExternalInput")
    w_w1bd = nc.dram_tensor("w_w1bd", [G1, G1], f32, kind="ExternalInput")
    w_rootw2 = nc.dram_tensor("w_rootw2", [HID_C, G2], f32, kind="ExternalInput")
    w_b2 = nc.dram_tensor("w_b2", [1, G2], f32, kind="ExternalInput")
    w_initw2 = nc.dram_tensor("w_initw2", [HID_C, G2], f32, kind="ExternalInput")
    w_w2bd = nc.dram_tensor("w_w2bd", [G2, G2], f32, kind="ExternalInput")
    out_d = nc.dram_tensor("out", [NPC, OUT_C], f32, kind="ExternalOutput")

    # ---- internal DRAM ----
    y = {
        1: nc.dram_tensor("y1", [NREP, IN_C], f32, addr_space="Shared"),
        2: nc.dram_tensor("y2", [NREP, G1], f32, addr_space="Shared"),
        3: nc.dram_tensor("y3", [NREP, HID_C], f32, addr_space="Shared"),
        4: nc.dram_tensor("y4", [NREP, G2], f32, addr_space="Shared"),
    }
    ag_in = {
        1: nc.dram_tensor("agin1", [NPC, IN_C], f32),
        2: nc.dram_tensor("agin2", [NPC, G1], f32),
        3: nc.dram_tensor("agin3", [NPC, HID_C], f32),
        4: nc.dram_tensor("agin4", [NPC, G2], f32),
    }
    FW = {1: IN_C, 2: G1, 3: HID_C, 4: G2}
    GW = {1: G1, 2: G1, 3: G2, 4: G2}

    rg = [list(range(cfg.CORES))]

    max_gt_elems = max(
        max((st.col_off[b1] - st.col_off[b0]) * FW[r]
            for (b0, b1) in st.groups[r])
        for r in (1, 2, 3, 4)
    )

    dsem = nc.alloc_semaphore("gsem")
    n_dma = [0]  # cumulative dma_gather count (crit mode)
    NSEM = 8
    dsems = [nc.alloc_semaphore(f"gsem{i}") for i in range(NSEM)]
    sem_count = [0] * NSEM
    gidx = [0]  # global group counter
    wait_tgt = [None]  # (sem, value) the current group's consumers wait on

    with tile.TileContext(nc) as tc:
        with (
            tc.tile_pool(name="const", bufs=1) as cpool,
            tc.tile_pool(name="gather", bufs=3) as gpool,
            tc.tile_pool(name="work", bufs=3) as wpool,
            tc.tile_pool(name="psum", bufs=3, space="PSUM") as ppool,
        ):
            # ---------- constants ----------
            ident = cpool.tile([P, P], f32)
            make_identity(nc, ident[:])

            def load_w(t, shape, tag):
                s = cpool.tile(list(shape), f32, tag=tag)
                nc.sync.dma_start(out=s[:], in_=t[:, :])
                return s

            rootw1_s = load_w(w_rootw1, (IN_C, G1), "w_rootw1")
            initw1_s = load_w(w_initw1, (IN_C, G1), "w_initw1")
            w1bd_s = load_w(w_w1bd, (G1, G1), "w_w1bd")
            rootw2_s = load_w(w_rootw2, (HID_C, G2), "w_rootw2")
            initw2_s = load_w(w_initw2, (HID_C, G2), "w_initw2")
            w2bd_s = load_w(w_w2bd, (G2, G2), "w_w2bd")
            b1_s = load_w(w_b1, (1, G1), "w_b1")
            b2_s = load_w(w_b2, (1, G2), "w_b2")
            rhs_s = {1: initw1_s, 2: w1bd_s, 3: initw2_s, 4: w2bd_s}

            ones1 = cpool.tile([1, P], f32)
            nc.vector.memset(ones1[:], 1.0)
            b1rep = cpool.tile([P, G1], f32)
            b2rep = cpool.tile([P, G2], f32)
            bps = ppool.tile([P, G1], f32, tag="mmps")
            nc.tensor.matmul(bps[:], lhsT=ones1[:], rhs=b1_s[:], start=True, stop=True)
            nc.vector.tensor_copy(b1rep[:], bps[:])
            bps2 = ppool.tile([P, G2], f32, tag="mmps")
            nc.tensor.matmul(bps2[:], lhsT=ones1[:], rhs=b2_s[:], start=True, stop=True)
            nc.vector.tensor_copy(b2rep[:], bps2[:])

            # ---------- gather indices ----------
            idx16_s = cpool.tile([P, WTOT], i16)
            nc.sync.dma_start(out=idx16_s[:], in_=idx16_d[:, :])

            # ---------- degrees (from the -1-padded int32 helper) ----------
            root1 = cpool.tile([P, NB, G1], f32)
            root2 = cpool.tile([P, NB, G2], f32)
            dinv = cpool.tile([P, NB], f32)
            dinvh = cpool.tile([P, NB], f32)
            with tc.tile_pool(name="prolog", bufs=1) as qpool:
                idx32_s = qpool.tile([P, st.tot_cols], i32)
                nc.sync.dma_start(out=idx32_s[:], in_=idx32_d[:, :])
                idxf = qpool.tile([P, st.tot_cols], f32)
                nc.vector.tensor_copy(idxf[:], idx32_s[:])
                valid = qpool.tile([P, st.tot_cols], f32)
                nc.vector.tensor_single_scalar(
                    valid[:], idxf[:], -1.0, Alu.not_equal
                )
                deg = qpool.tile([P, NB], f32)
                for b in range(NB):
                    c0, c1 = st.col_off[b], st.col_off[b + 1]
                    nc.vector.reduce_sum(deg[:, b:b + 1], valid[:, c0:c1], axis=X)
                degc = qpool.tile([P, NB], f32)
                nc.vector.tensor_scalar_max(degc[:], deg[:], 1.0)
                sq = qpool.tile([P, NB], f32)
                nc.scalar.activation(sq[:], degc[:], Act.Sqrt)
                rinv = qpool.tile([P, NB], f32)
                nc.vector.reciprocal(rinv[:], sq[:])
                mask = qpool.tile([P, NB], f32)
                nc.vector.tensor_single_scalar(mask[:], deg[:], 0.0, Alu.is_gt)
                nc.vector.tensor_mul(dinv[:], rinv[:], mask[:])
                nc.vector.tensor_scalar_mul(dinvh[:], dinv[:], 0.5)

                # ---------- roots + Y1 ----------
                x_s = qpool.tile([P, NB, IN_C], f32)
                for b in range(NB):
                    nc.sync.dma_start(
                        out=x_s[:, b, :], in_=xs[b * P:(b + 1) * P, :]
                    )
                for b in range(NB):
                    dcol = dinv[:, b:b + 1]
                    xT_ps = ppool.tile([IN_C, P], f32, tag="tps")
                    nc.tensor.transpose(xT_ps[:], x_s[:, b, :], ident[:])
                    xT = wpool.tile([IN_C, P], f32, tag="aggT")
                    nc.scalar.activation(xT[:], xT_ps[:], Act.Copy)
                    r1_ps = ppool.tile([P, G1], f32, tag="mmps")
                    nc.tensor.matmul(
                        r1_ps[:], lhsT=xT[:], rhs=rootw1_s[:], start=True, stop=True
                    )
                    nc.vector.tensor_add(root1[:, b, :], r1_ps[:], b1rep[:])
                    y1b = wpool.tile([P, IN_C], f32, tag="yout")
                    nc.scalar.activation(y1b[:], x_s[:, b, :], Act.Copy, scale=dcol)
                    nc.sync.dma_start(
                        out=ag_in[1][b * P:(b + 1) * P, :], in_=y1b[:]
                    )
            import os as _os
            if not _os.environ.get("GNN_SKIP_AG0"):
                nc.gpsimd.collective_compute(
                    "AllGather", Alu.bypass, replica_groups=rg,
                    ins=[ag_in[1].ap().opt()], outs=[y[1].ap().opt()],
                )

            # ---------- 4 message-passing rounds ----------
            import os
            max_round = int(os.environ.get("GNN_STAGE", "4"))
            reps = int(os.environ.get("GNN_REPS", "1"))

            def gather_group(r, b0, b1):
                """Issue the A and B dma_gather calls for blocks [b0,b1);
                returns the gather tile (cols: A of b0..b1, then B)."""
                F = FW[r]
                nA = (a_cum[b1] - a_cum[b0]) * P
                nB = (b_cum[b1] - b_cum[b0]) * P
                if os.environ.get("GNN_TINY_GATHER"):
                    # timing experiment: same call count, 128 idx each
                    gt = gpool.tile([P, max_gt_elems], f32, tag="gt")
                    s = gidx[0] % NSEM
                    gidx[0] += 1
                    sem_count[s] += 32
                    nc.gpsimd.dma_gather(
                        out_ap=gt[:, :F].rearrange("p (c f) -> p c f", f=F),
                        in_ap=y[r].ap(), idxs_ap=idx16_s[:, :8],
                        num_idxs=P, num_idxs_reg=P, elem_size=F,
                        single_packet=False,
                    ).then_inc(dsems[s], 16)
                    nc.gpsimd.dma_gather(
                        out_ap=gt[:, F:2 * F].rearrange("p (c f) -> p c f", f=F),
                        in_ap=y[r].ap(), idxs_ap=idx16_s[:, :8],
                        num_idxs=P, num_idxs_reg=P, elem_size=F,
                        single_packet=False,
                    ).then_inc(dsems[s], 16)
                    wait_tgt[0] = (dsems[s], sem_count[s])
                    return gt, (a_cum[b1] - a_cum[b0])
                ncols = (nA + nB) // P
                gt = gpool.tile([P, max_gt_elems], f32, tag="gt")
                outA = gt[:, :nA // P * F].rearrange("p (c f) -> p c f", f=F)
                outB = gt[:, nA // P * F:ncols * F].rearrange(
                    "p (c f) -> p c f", f=F
                )
                ixA = idx16_s[:, a_cum[b0] * 8:a_cum[b1] * 8]
                ixB = idx16_s[:, (TA + b_cum[b0]) * 8:(TA + b_cum[b1]) * 8]
                yv = y[r]
                if os.environ.get("GNN_SYNC_CRIT"):
                    with tc.tile_critical():
                        nc.gpsimd.dma_gather(
                            out_ap=outA, in_ap=yv.ap(), idxs_ap=ixA,
                            num_idxs=nA, num_idxs_reg=nA, elem_size=F,
                            single_packet=False,
                        ).then_inc(dsem, 16)
                        nc.gpsimd.dma_gather(
                            out_ap=outB,
                            in_ap=yv[HALF:NREP, :],
                            idxs_ap=ixB,
                            num_idxs=nB, num_idxs_reg=nB, elem_size=F,
                            single_packet=False,
                        ).then_inc(dsem, 16)
                        n_dma[0] += 2
                        nc.gpsimd.wait_ge(dsem, 16 * n_dma[0])
                else:
                    s = gidx[0] % NSEM
                    gidx[0] += 1
                    sem_count[s] += 32
                    nc.gpsimd.dma_gather(
                        out_ap=outA, in_ap=yv.ap(), idxs_ap=ixA,
                        num_idxs=nA, num_idxs_reg=nA, elem_size=F,
                        single_packet=False,
                    ).then_inc(dsems[s], 16)
                    nc.gpsimd.dma_gather(
                        out_ap=outB,
                        in_ap=yv[HALF:NREP, :],
                        idxs_ap=ixB,
                        num_idxs=nB, num_idxs_reg=nB, elem_size=F,
                        single_packet=False,
                    ).then_inc(dsems[s], 16)
                    wait_tgt[0] = (dsems[s], sem_count[s])
                return gt, nA // P

            for rep in range(reps):
              for r in (1, 2, 3, 4):
                if r > max_round:
                    break
                F = FW[r]
                G = GW[r]
                for (b0, b1) in st.groups[r]:
                    gt, colsA = gather_group(r, b0, b1)
                    if os.environ.get("GNN_R1_LITE"):
                        lite_t = wpool.tile([P, FW[r]], f32, tag="agg")
                        nc.vector.tensor_copy(lite_t[:], gt[:, :FW[r]])
                        continue
                    for b in range(b0, b1):
                        dcol = dinv[:, b:b + 1]
                        # A slots at cols [a_cum[b]-a_cum[b0], +DA[b]),
                        # B slots at colsA + [b_cum[b]-b_cum[b0], +DB[b])
                        oA = a_cum[b] - a_cum[b0]
                        oB = colsA + (b_cum[b] - b_cum[b0])
                        aggA = wpool.tile([P, F], f32, tag="aggA")
                        rA = nc.vector.reduce_sum(
                            aggA[:],
                            gt[:, oA * F:(oA + DA[b]) * F].rearrange(
                                "p (d f) -> p f d", f=F
                            ),
                            axis=X,
                        )
                        aggB = wpool.tile([P, F], f32, tag="aggB")
                        rB = nc.vector.reduce_sum(
                            aggB[:],
                            gt[:, oB * F:(oB + DB[b]) * F].rearrange(
                                "p (d f) -> p f d", f=F
                            ),
                            axis=X,
                        )
                        if not os.environ.get("GNN_SYNC_CRIT"):
                            ws, wv = wait_tgt[0]
                            rA._wait_ge(ws, wv)
                            rB._wait_ge(ws, wv)
                        agg = wpool.tile([P, F], f32, tag="agg")
                        nc.vector.tensor_add(agg[:], aggA[:], aggB[:])
                        aggT_ps = ppool.tile([F, P], f32, tag="tps")
                        nc.tensor.transpose(aggT_ps[:], agg[:], ident[:])
                        aggT = wpool.tile([F, P], f32, tag="aggT")
                        nc.scalar.activation(aggT[:], aggT_ps[:], Act.Copy)
                        mm_ps = ppool.tile([P, G], f32, tag="mmps")
                        nc.tensor.matmul(
                            mm_ps[:], lhsT=aggT[:], rhs=rhs_s[r][:],
                            start=True, stop=True,
                        )
                        root = root1 if r <= 2 else root2
                        t_sb = wpool.tile([P, G], f32, tag="tsb")
                        nc.vector.scalar_tensor_tensor(
                            t_sb[:], mm_ps[:], dcol, root[:, b, :],
                            op0=Alu.mult, op1=Alu.add,
                        )
                        if r == 1:
                            yo = wpool.tile([P, G1], f32, tag="yout")
                            nc.scalar.activation(yo[:], t_sb[:], Act.Relu, scale=dcol)
                            nc.sync.dma_start(
                                out=ag_in[2][b * P:(b + 1) * P, :], in_=yo[:]
                            )
                        elif r == 2:
                            out1 = wpool.tile([P, G1], f32, tag="out1")
                            nc.scalar.activation(out1[:], t_sb[:], Act.Relu)
                            hsum = wpool.tile([P, HID_C], f32, tag="hsum")
                            nc.vector.tensor_add(
                                hsum[:], out1[:, :HID_C], out1[:, HID_C:]
                            )
                            yo = wpool.tile([P, HID_C], f32, tag="yout")
                            nc.scalar.activation(
                                yo[:], hsum[:], Act.Copy, scale=dinvh[:, b:b + 1]
                            )
                            nc.sync.dma_start(
                                out=ag_in[3][b * P:(b + 1) * P, :], in_=yo[:]
                            )
                            hT_ps = ppool.tile([HID_C, P], f32, tag="tps")
                            nc.tensor.transpose(hT_ps[:], hsum[:], ident[:])
                            hT = wpool.tile([HID_C, P], f32, tag="aggT")
                            nc.scalar.activation(hT[:], hT_ps[:], Act.Copy)
                            r2_ps = ppool.tile([P, G2], f32, tag="mmps")
                            nc.tensor.matmul(
                                r2_ps[:], lhsT=hT[:], rhs=rootw2_s[:],
                                start=True, stop=True,
                            )
                            nc.vector.tensor_add(root2[:, b, :], r2_ps[:], b2rep[:])
                        elif r == 3:
                            yo = wpool.tile([P, G2], f32, tag="yout")
                            nc.scalar.activation(yo[:], t_sb[:], Act.Relu, scale=dcol)
                            nc.sync.dma_start(
                                out=ag_in[4][b * P:(b + 1) * P, :], in_=yo[:]
                            )
                        else:
                            ofin = wpool.tile([P, G2], f32, tag="out1")
                            nc.scalar.activation(ofin[:], t_sb[:], Act.Relu)
                            msum = wpool.tile([P, OUT_C], f32, tag="hsum")
                            nc.vector.tensor_add(
                                msum[:], ofin[:, :OUT_C], ofin[:, OUT_C:]
                            )
                            yo = wpool.tile([P, OUT_C], f32, tag="yout")
                            nc.scalar.activation(yo[:], msum[:], Act.Copy, scale=0.5)
                            nc.sync.dma_start(
                                out=out_d[b * P:(b + 1) * P, :], in_=yo[:]
                            )
                if r < 4 and r < max_round and not os.environ.get("GNN_SKIP_AG"):
                    nc.gpsimd.collective_compute(
                        "AllGather", Alu.bypass, replica_groups=rg,
                        ins=[ag_in[r + 1].ap().opt()], outs=[y[r + 1].ap().opt()],
                    )

    nc.compile()
    return nc


def build_in_maps(inputs: dict, cfg: Cfg, st: Struct) -> list:
    x = np.asarray(inputs["x"], dtype=np.float32)
    wmap = build_weight_inputs(inputs, cfg)
    in_maps = []
    for c in range(cfg.CORES):
        xs = np.zeros((cfg.NPC, cfg.IN_C), np.float32)
        mine = np.nonzero(st.pid // cfg.NPC == c)[0]
        loc = st.pid[mine] % cfg.NPC
        xs[loc] = x[mine]
        m = {
            "xs": xs,
            "idx16": np.ascontiguousarray(st.idx16[c]),
            "idx32": np.ascontiguousarray(st.idx32[c]),
        }
        m.update(wmap)
        in_maps.append(m)
    return in_maps


def assemble_output(results: list, cfg: Cfg, st: Struct) -> np.ndarray:
    full = np.concatenate(
        [np.asarray(results[c]["out"]) for c in range(cfg.CORES)], axis=0
    )
    return np.ascontiguousarray(full[st.pid]).astype(np.float32)


def kernel(**inputs) -> np.ndarray:
    from concourse.bass_utils import run_bass_kernel_spmd

    cfg = Cfg()
    st = build_structure(np.asarray(inputs["edge_index"]), cfg)
    nc = build_nc(cfg, st)
    in_maps = build_in_maps(inputs, cfg, st)
    res = run_bass_kernel_spmd(nc, in_maps, core_ids=list(range(cfg.CORES)))
    return assemble_output(res.results, cfg, st)


if __name__ == "__main__":
    pass



# revision 4
# speedup vs baseline: 11.0665x; 3.4518x over previous
"""Trainium2 Bass kernel for nn_BiARMA (2-layer ARMA GNN, K=2 stacks, T=2).

Math: A = D^-1/2 C D^-1/2 (C = edge-count matrix, deg by dst).
Key identity: norm[e] = dinv[src]*dinv[dst] factors, so
  segment_sum(out[src]*norm, dst) = dinv_dst * gather_sum(dinv_src*out[src])
-> every message-passing round is a pure row-gather-accumulate of a
pre-scaled node tensor.  Weights commute with aggregation, so matmuls
run on the aggregated tensor.

Distribution: dst-nodes sharded over 8 cores (graph parallel).  Each
core keeps a full replica of the current pre-scaled node tensor in its
DRAM, gathers rows for its local edges with the gpsimd dma_gather
ucode op, reduces padded per-node slots on DVE, applies weights on PE,
and AllGathers its updated shard each round.

Gather indices are int16 (<=32768 addressable rows), but the replica
has 50176 rows.  Two gather calls per group with OVERLAPPING windows:
call A reads rows [0, 32768) and call B reads rows [17408, 50176) of
the same replica.  Sources on cores 0-2 must use call A, cores 5-7
call B, and cores 3-4 (filled with the highest out-degree nodes) can
use either -- those flexible edges are assigned per destination to
balance the two calls, which makes the per-block padded slot counts
(max over cores x 128 partition rows) nearly tight: ~899 padded slot
columns vs 782 ideal vs 1387 for a disjoint-half split.
"""

import os
import sys
from dataclasses import dataclass, field

import numpy as np

sys.path.insert(0, "/opt/trn_rl_repo")

P = 128
WIN = 32768  # rows addressable by one int16-indexed gather call


@dataclass
class Cfg:
    N: int = 50000
    E: int = 800000
    IN_C: int = 64
    HID_C: int = 64
    OUT_C: int = 32
    K: int = 2
    CORES: int = 8
    # gather-tile budget, elements per partition per group (per dtype)
    group_budget_elems: int = 6144

    @property
    def blocks(self):
        return (self.N // self.CORES + 1 + P - 1) // P

    @property
    def NPC(self):
        return self.blocks * P

    @property
    def NREP(self):
        return self.CORES * self.NPC

    @property
    def OFFB(self):  # window-B base row
        return self.NREP - WIN


@dataclass
class Struct:
    DA: list
    DB: list
    col_off: list       # per-block column offset (A+B combined)
    a_off: list         # per-block offset into the A column space
    b_off: list         # per-block offset into the B column space
    tot_cols: int
    idx16: np.ndarray   # [CORES, 128, (TA+TB)*8] int16 wrapped+replicated
    idx32: np.ndarray   # [CORES, P, tot_cols] int32, -1 padded (deg helper)
    pid: np.ndarray
    a_cum: list = None
    b_cum: list = None
    TA: int = 0
    TB: int = 0
    groups: dict = field(default_factory=dict)


def build_structure(edge_index: np.ndarray, cfg: Cfg) -> Struct:
    src = np.asarray(edge_index[0], dtype=np.int64)
    dst = np.asarray(edge_index[1], dtype=np.int64)
    N, CORES, NPC, NB = cfg.N, cfg.CORES, cfg.NPC, cfg.blocks
    OFFB = cfg.OFFB

    # ---- core assignment: highest out-degree nodes fill cores 3,4 (the
    # cores whose pid range lies inside BOTH gather windows), everything
    # else round-robins over the remaining cores ----
    outdeg = np.bincount(src, minlength=N)
    o = np.argsort(-outdeg, kind="stable")
    core_of = np.empty(N, np.int64)
    nflex = 2 * NPC
    core_of[o[:nflex]] = np.where(np.arange(nflex) % 2 == 0, 3, 4)
    rest_cores = np.array([0, 1, 2, 5, 6, 7])
    core_of[o[nflex:]] = rest_cores[np.arange(N - nflex) % 6]

    # ---- call assignment per edge: by source core, flexible edges
    # (src on cores 3,4) greedily balance each destination's two calls ----
    sc = core_of[src]
    a_fixed = sc <= 2
    b_fixed = sc >= 5
    cA = np.zeros(N, np.int64)
    cB = np.zeros(N, np.int64)
    np.add.at(cA, dst[a_fixed], 1)
    np.add.at(cB, dst[b_fixed], 1)
    half = np.where(a_fixed, 0, 1).astype(np.int64)
    fidx = np.nonzero(~a_fixed & ~b_fixed)[0]
    fidx = fidx[np.argsort(dst[fidx], kind="stable")]
    fd = dst[fidx]
    # per-dst greedy balance, vectorized: within each dst's flexible run,
    # first |imb| edges go to the lighter side, then alternate
    runs = np.concatenate([[0], np.cumsum(np.bincount(fd, minlength=N))])
    pos = np.arange(fidx.shape[0]) - runs[fd]
    imb = (cA - cB)[fd]  # >0: A heavier -> first flex edges go to B
    rem = pos - np.abs(imb)
    go_b = np.where(rem < 0, imb > 0, (rem % 2 == 1) ^ (imb < 0))
    half[fidx] = go_b.astype(np.int64)
    cA = np.zeros(N, np.int64)
    cB = np.zeros(N, np.int64)
    np.add.at(cA, dst[half == 0], 1)
    np.add.at(cB, dst[half == 1], 1)

    # ---- local ordering within each core: lexicographic (cA, cB) desc
    # -> tight per-block padded widths ----
    local_of = np.empty(N, np.int64)
    for c in range(CORES):
        nodes = np.nonzero(core_of == c)[0]
        o2 = np.lexsort((-cB[nodes], -cA[nodes]))
        local_of[nodes[o2]] = np.arange(len(nodes))
    pid = core_of * NPC + local_of

    # window sanity: A-call sources have pid < WIN, B-call >= OFFB
    spid = pid[src]
    assert spid[half == 0].max() < WIN
    assert spid[half == 1].min() >= OFFB

    ecore = core_of[dst]
    dloc = local_of[dst]
    sloc = np.where(half == 0, spid, spid - OFFB)  # window-local index

    # per (call, core, node) counts -> per-block padded A/B widths
    cnt = np.zeros((2, CORES, NPC), np.int64)
    for h in (0, 1):
        for c in range(CORES):
            m = (ecore == c) & (half == h)
            cnt[h, c] = np.bincount(dloc[m], minlength=NPC)
    DA = cnt[0].reshape(CORES, NB, P).max(axis=(0, 2))
    DB = cnt[1].reshape(CORES, NB, P).max(axis=(0, 2))
    DA = np.maximum(DA, 1).tolist()
    DB = np.maximum(DB, 1).tolist()
    D = [DA[b] + DB[b] for b in range(NB)]
    col_off = np.concatenate([[0], np.cumsum(D)]).tolist()
    a_off = [col_off[b] for b in range(NB)]          # A slots first per block
    b_off = [col_off[b] + DA[b] for b in range(NB)]  # then B slots
    tot_cols = int(col_off[-1])

    # per-slot values, node-major layout [P, tot_cols]
    vals = np.full((CORES, P, tot_cols), -1, np.int64)
    eo = np.lexsort((dloc, ecore))
    ecore_s, dloc_s, sloc_s, half_s = ecore[eo], dloc[eo], sloc[eo], half[eo]
    aoff = np.asarray(a_off)
    boff = np.asarray(b_off)
    for c in range(CORES):
        m = ecore_s == c
        dl, sl, hh = dloc_s[m], sloc_s[m], half_s[m]
        for h in (0, 1):
            mh = hh == h
            dlh, slh = dl[mh], sl[mh]
            cth = np.bincount(dlh, minlength=NPC)
            starts = np.concatenate([[0], np.cumsum(cth)])[:-1]
            p_in = np.arange(dlh.shape[0]) - starts[dlh]
            b = dlh // P
            p = dlh % P
            col = (aoff if h == 0 else boff)[b] + p_in
            vals[c, p, col] = slh

    # int32 deg helper (-1 = pad)
    idx32 = vals.astype(np.int32)

    st = Struct(DA=DA, DB=DB, col_off=col_off, a_off=a_off, b_off=b_off,
                tot_cols=tot_cols, idx16=None, idx32=idx32, pid=pid)

    def make_groups(budget_cols):
        budget = max(budget_cols, max(D))
        groups = []
        b0 = 0
        while b0 < NB:
            b1 = b0
            tot = 0
            while b1 < NB and (tot + D[b1] <= budget or b1 == b0):
                tot += D[b1]
                b1 += 1
            groups.append((b0, b1))
            b0 = b1
        return groups

    be = cfg.group_budget_elems
    st.groups = {
        1: make_groups(be // cfg.IN_C),
        2: make_groups(be // (cfg.K * cfg.HID_C)),
        3: make_groups(be // cfg.HID_C),
        4: make_groups(be // (cfg.K * cfg.OUT_C)),
    }

    # int16 gather arrays.  For ANY contiguous block range, call A reads
    # the A-columns of blocks b0..b1 in block order, call B the
    # B-columns.  Store two wrapped arrays (all A-columns block-major,
    # then all B-columns) so every call's index slice is contiguous.
    # Flat call position i -> (partition i%128, out column i//128);
    # wrapped storage (i%16, i//16), replicated x8 to 128 partitions.
    a_cum = np.concatenate([[0], np.cumsum(DA)]).astype(np.int64)
    b_cum = np.concatenate([[0], np.cumsum(DB)]).astype(np.int64)
    TA, TB = int(a_cum[-1]), int(b_cum[-1])
    st.a_cum = a_cum.tolist()
    st.b_cum = b_cum.tolist()
    st.TA, st.TB = TA, TB

    # pad rows: phantom (always-zero) rows inside each window
    PADA = NPC - 1               # core 0, top local -- phantom
    PADB = CORES * NPC - 1 - OFFB  # core 7, top local, window-B-relative
    assert PADA < WIN and 0 <= PADB < WIN

    idx16 = np.empty((CORES, 16, (TA + TB) * 8), np.int16)
    for c in range(CORES):
        va = np.empty((P, TA), np.int64)
        vb = np.empty((P, TB), np.int64)
        for b in range(NB):
            va[:, a_cum[b]:a_cum[b + 1]] = \
                vals[c][:, a_off[b]:a_off[b] + DA[b]]
            vb[:, b_cum[b]:b_cum[b + 1]] = \
                vals[c][:, b_off[b]:b_off[b] + DB[b]]
        va = np.where(va < 0, PADA, va)
        vb = np.where(vb < 0, PADB, vb)
        both = np.concatenate([va, vb], axis=1).astype(np.int16)
        # column c', partition p -> flat i = c'*128 + p -> (i%16, i//16):
        # wrapped[r, c'*8 + q] with p = q*16 + r
        w = both.reshape(16, 8, TA + TB, order="F")
        w2 = np.transpose(w, (0, 2, 1)).reshape(16, (TA + TB) * 8)
        idx16[c] = w2
    st.idx16 = np.tile(idx16, (1, 8, 1))
    return st


def build_weight_inputs(inp: dict, cfg: Cfg) -> dict:
    K, IN_C, HID_C, OUT_C = cfg.K, cfg.IN_C, cfg.HID_C, cfg.OUT_C
    f4 = lambda a: np.ascontiguousarray(a, dtype=np.float32)

    rootw1 = np.transpose(inp["root_w1"][0], (1, 0, 2)).reshape(IN_C, K * HID_C)
    b1row = inp["b1"][0, :, 0, :].reshape(1, K * HID_C)
    initw1 = np.transpose(inp["init_w1"], (1, 0, 2)).reshape(IN_C, K * HID_C)
    w1bd = np.zeros((K * HID_C, K * HID_C), np.float32)
    for k in range(K):
        w1bd[k * HID_C:(k + 1) * HID_C, k * HID_C:(k + 1) * HID_C] = inp["w1"][0, k]

    # 0.5 absorbed: round-2 h-stage feeds the UNhalved stack sum into root2
    rootw2 = 0.5 * np.transpose(inp["root_w2"][0], (1, 0, 2)).reshape(HID_C, K * OUT_C)
    b2row = inp["b2"][0, :, 0, :].reshape(1, K * OUT_C)
    initw2 = np.transpose(inp["init_w2"], (1, 0, 2)).reshape(HID_C, K * OUT_C)
    w2bd = np.zeros((K * OUT_C, K * OUT_C), np.float32)
    for k in range(K):
        w2bd[k * OUT_C:(k + 1) * OUT_C, k * OUT_C:(k + 1) * OUT_C] = inp["w2"][0, k]

    return {
        "w_rootw1": f4(rootw1), "w_b1": f4(b1row), "w_initw1": f4(initw1),
        "w_w1bd": f4(w1bd), "w_rootw2": f4(rootw2), "w_b2": f4(b2row),
        "w_initw2": f4(initw2), "w_w2bd": f4(w2bd),
    }


def build_nc(cfg: Cfg, st: Struct):
    import concourse.bacc as bacc
    import concourse.bass as bass
    import concourse.mybir as mybir
    import concourse.tile as tile
    from concourse.masks import make_identity

    f32 = mybir.dt.float32
    bf16 = mybir.dt.bfloat16
    i16 = mybir.dt.int16
    i32 = mybir.dt.int32
    X = mybir.AxisListType.X
    Alu = mybir.AluOpType
    Act = mybir.ActivationFunctionType

    K, IN_C, HID_C, OUT_C = cfg.K, cfg.IN_C, cfg.HID_C, cfg.OUT_C
    G1 = K * HID_C   # 128
    G2 = K * OUT_C   # 64
    NB = cfg.blocks
    NPC, NREP, OFFB = cfg.NPC, cfg.NREP, cfg.OFFB
    DA, DB = st.DA, st.DB
    TA, TB = st.TA, st.TB
    a_cum, b_cum = st.a_cum, st.b_cum
    WTOT = (TA + TB) * 8

    nc = bacc.Bacc(
        "TRN2",
        target_bir_lowering=False,
        debug=False,
        num_devices=cfg.CORES,
    )

    # ---- kernel I/O ----
    xs = nc.dram_tensor("xs", [NPC, IN_C], f32, kind="ExternalInput")
    idx16_d = nc.dram_tensor("idx16", [P, WTOT], i16, kind="ExternalInput")
    idx32_d = nc.dram_tensor("idx32", [P, st.tot_cols], i32, kind="ExternalInput")
    w_rootw1 = nc.dram_tensor("w_rootw1", [IN_C, G1], f32, kind="ExternalInput")
    w_b1 = nc.dram_tensor("w_b1", [1, G1], f32, kind="ExternalInput")
    w_initw1 = nc.dram_tensor("w_initw1", [IN_C, G1], f32, kind="ExternalInput")
    w_w1bd = nc.dram_tensor("w_w1bd", [G1, G1], f32, kind="ExternalInput")
    w_rootw2 = nc.dram_tensor("w_rootw2", [HID_C, G2], f32, kind="ExternalInput")
    w_b2 = nc.dram_tensor("w_b2", [1, G2], f32, kind="ExternalInput")
    w_initw2 = nc.dram_tensor("w_initw2", [HID_C, G2], f32, kind="ExternalInput")
    w_w2bd = nc.dram_tensor("w_w2bd", [G2, G2], f32, kind="ExternalInput")
    out_d = nc.dram_tensor("out", [NPC, OUT_C], f32, kind="ExternalOutput")

    # ---- internal DRAM (y2 in bf16: halves AllGather + gather bytes,
    # 128 bf16 = 256B descriptors still satisfy the gather constraint) ----
    y = {
        1: nc.dram_tensor("y1", [NREP, IN_C], f32, addr_space="Shared"),
        2: nc.dram_tensor("y2", [NREP, G1], bf16, addr_space="Shared"),
        3: nc.dram_tensor("y3", [NREP, HID_C], f32, addr_space="Shared"),
        4: nc.dram_tensor("y4", [NREP, G2], f32, addr_space="Shared"),
    }
    ag_in = {
        1: nc.dram_tensor("agin1", [NPC, IN_C], f32),
        2: nc.dram_tensor("agin2", [NPC, G1], bf16),
        3: nc.dram_tensor("agin3", [NPC, HID_C], f32),
        4: nc.dram_tensor("agin4", [NPC, G2], f32),
    }
    FW = {1: IN_C, 2: G1, 3: HID_C, 4: G2}
    GW = {1: G1, 2: G1, 3: G2, 4: G2}
    YDT = {1: f32, 2: bf16, 3: f32, 4: f32}

    rg = [list(range(cfg.CORES))]

    max_gt = {
        dt: max(
            (max((st.col_off[b1] - st.col_off[b0]) * FW[r]
                 for (b0, b1) in st.groups[r])
             for r in (1, 2, 3, 4) if YDT[r] == dt),
            default=0,
        )
        for dt in (f32, bf16)
    }

    NSEM = 8
    dsems = [nc.alloc_semaphore(f"gsem{i}") for i in range(NSEM)]
    sem_count = [0] * NSEM
    gidx = [0]  # global gather-call counter
    wait_a = [None]
    wait_b = [None]

    with tile.TileContext(nc) as tc:
        with (
            tc.tile_pool(name="const", bufs=1) as cpool,
            tc.tile_pool(name="gather", bufs=3) as gpool,
            tc.tile_pool(name="work", bufs=3) as wpool,
            tc.tile_pool(name="psum", bufs=3, space="PSUM") as ppool,
        ):
            # ---------- constants ----------
            ident = cpool.tile([P, P], f32)
            make_identity(nc, ident[:])

            def load_w(t, shape, tag):
                s = cpool.tile(list(shape), f32, tag=tag)
                nc.sync.dma_start(out=s[:], in_=t[:, :])
                return s

            rootw1_s = load_w(w_rootw1, (IN_C, G1), "w_rootw1")
            initw1_s = load_w(w_initw1, (IN_C, G1), "w_initw1")
            w1bd_s = load_w(w_w1bd, (G1, G1), "w_w1bd")
            rootw2_s = load_w(w_rootw2, (HID_C, G2), "w_rootw2")
            initw2_s = load_w(w_initw2, (HID_C, G2), "w_initw2")
            w2bd_s = load_w(w_w2bd, (G2, G2), "w_w2bd")
            b1_s = load_w(w_b1, (1, G1), "w_b1")
            b2_s = load_w(w_b2, (1, G2), "w_b2")
            rhs_s = {1: initw1_s, 2: w1bd_s, 3: initw2_s, 4: w2bd_s}

            ones1 = cpool.tile([1, P], f32)
            nc.vector.memset(ones1[:], 1.0)
            b1rep = cpool.tile([P, G1], f32)
            b2rep = cpool.tile([P, G2], f32)
            bps = ppool.tile([P, G1], f32, tag="mmps")
            nc.tensor.matmul(bps[:], lhsT=ones1[:], rhs=b1_s[:], start=True, stop=True)
            nc.vector.tensor_copy(b1rep[:], bps[:])
            bps2 = ppool.tile([P, G2], f32, tag="mmps")
            nc.tensor.matmul(bps2[:], lhsT=ones1[:], rhs=b2_s[:], start=True, stop=True)
            nc.vector.tensor_copy(b2rep[:], bps2[:])

            # ---------- gather indices ----------
            idx16_s = cpool.tile([P, WTOT], i16)
            nc.sync.dma_start(out=idx16_s[:], in_=idx16_d[:, :])

            # ---------- degrees (from the -1-padded int32 helper) ----------
            root1 = cpool.tile([P, NB, G1], f32)
            root2 = cpool.tile([P, NB, G2], f32)
            dinv = cpool.tile([P, NB], f32)
            dinvh = cpool.tile([P, NB], f32)
            with tc.tile_pool(name="prolog", bufs=1) as qpool:
                idx32_s = qpool.tile([P, st.tot_cols], i32)
                nc.sync.dma_start(out=idx32_s[:], in_=idx32_d[:, :])
                idxf = qpool.tile([P, st.tot_cols], f32)
                nc.vector.tensor_copy(idxf[:], idx32_s[:])
                valid = qpool.tile([P, st.tot_cols], f32)
                nc.vector.tensor_single_scalar(
                    valid[:], idxf[:], -1.0, Alu.not_equal
                )
                deg = qpool.tile([P, NB], f32)
                for b in range(NB):
                    c0, c1 = st.col_off[b], st.col_off[b + 1]
                    nc.vector.reduce_sum(deg[:, b:b + 1], valid[:, c0:c1], axis=X)
                degc = qpool.tile([P, NB], f32)
                nc.vector.tensor_scalar_max(degc[:], deg[:], 1.0)
                sq = qpool.tile([P, NB], f32)
                nc.scalar.activation(sq[:], degc[:], Act.Sqrt)
                rinv = qpool.tile([P, NB], f32)
                nc.vector.reciprocal(rinv[:], sq[:])
                mask = qpool.tile([P, NB], f32)
                nc.vector.tensor_single_scalar(mask[:], deg[:], 0.0, Alu.is_gt)
                nc.vector.tensor_mul(dinv[:], rinv[:], mask[:])
                nc.vector.tensor_scalar_mul(dinvh[:], dinv[:], 0.5)

                # ---------- roots + Y1 ----------
                x_s = qpool.tile([P, NB, IN_C], f32)
                for b in range(NB):
                    nc.sync.dma_start(
                        out=x_s[:, b, :], in_=xs[b * P:(b + 1) * P, :]
                    )
                for b in range(NB):
                    dcol = dinv[:, b:b + 1]
                    xT_ps = ppool.tile([IN_C, P], f32, tag="tps")
                    nc.tensor.transpose(xT_ps[:], x_s[:, b, :], ident[:])
                    xT = wpool.tile([IN_C, P], f32, tag="aggT")
                    nc.scalar.activation(xT[:], xT_ps[:], Act.Copy)
                    r1_ps = ppool.tile([P, G1], f32, tag="mmps")
                    nc.tensor.matmul(
                        r1_ps[:], lhsT=xT[:], rhs=rootw1_s[:], start=True, stop=True
                    )
                    nc.vector.tensor_add(root1[:, b, :], r1_ps[:], b1rep[:])
                    y1b = wpool.tile([P, IN_C], f32, tag="yout")
                    nc.scalar.activation(y1b[:], x_s[:, b, :], Act.Copy, scale=dcol)
                    nc.sync.dma_start(
                        out=ag_in[1][b * P:(b + 1) * P, :], in_=y1b[:]
                    )
            if not os.environ.get("GNN_SKIP_AG0"):
                nc.gpsimd.collective_compute(
                    "AllGather", Alu.bypass, replica_groups=rg,
                    ins=[ag_in[1].ap().opt()], outs=[y[1].ap().opt()],
                )

            # ---------- 4 message-passing rounds ----------
            max_round = int(os.environ.get("GNN_STAGE", "4"))
            reps = int(os.environ.get("GNN_REPS", "1"))

            def gather_group(r, b0, b1):
                """Issue the A and B dma_gather calls for blocks [b0,b1);
                returns the gather tile (cols: A of b0..b1, then B)."""
                F = FW[r]
                dt = YDT[r]
                nA = (a_cum[b1] - a_cum[b0]) * P
                nB = (b_cum[b1] - b_cum[b0]) * P
                ncols = (nA + nB) // P
                gt = gpool.tile(
                    [P, max_gt[dt]], dt,
                    tag="gt32" if dt == f32 else "gt16",
                )
                outA = gt[:, :nA // P * F].rearrange("p (c f) -> p c f", f=F)
                outB = gt[:, nA // P * F:ncols * F].rearrange(
                    "p (c f) -> p c f", f=F
                )
                ixA = idx16_s[:, a_cum[b0] * 8:a_cum[b1] * 8]
                ixB = idx16_s[:, (TA + b_cum[b0]) * 8:(TA + b_cum[b1]) * 8]
                yv = y[r]
                sA = (2 * gidx[0]) % NSEM
                sB = (2 * gidx[0] + 1) % NSEM
                gidx[0] += 1
                sem_count[sA] += 16
                sem_count[sB] += 16
                nc.gpsimd.dma_gather(
                    out_ap=outA, in_ap=yv[0:WIN, :], idxs_ap=ixA,
                    num_idxs=nA, num_idxs_reg=nA, elem_size=F,
                    single_packet=False,
                ).then_inc(dsems[sA], 16)
                nc.gpsimd.dma_gather(
                    out_ap=outB, in_ap=yv[OFFB:NREP, :], idxs_ap=ixB,
                    num_idxs=nB, num_idxs_reg=nB, elem_size=F,
                    single_packet=False,
                ).then_inc(dsems[sB], 16)
                wait_a[0] = (dsems[sA], sem_count[sA])
                wait_b[0] = (dsems[sB], sem_count[sB])
                return gt, nA // P

            for rep in range(reps):
              for r in (1, 2, 3, 4):
                if r > max_round:
                    break
                F = FW[r]
                G = GW[r]
                for (b0, b1) in st.groups[r]:
                    gt, colsA = gather_group(r, b0, b1)
                    for b in range(b0, b1):
                        dcol = dinv[:, b:b + 1]
                        oA = a_cum[b] - a_cum[b0]
                        oB = colsA + (b_cum[b] - b_cum[b0])
                        aggA = wpool.tile([P, F], f32, tag="aggA")
                        rA = nc.vector.reduce_sum(
                            aggA[:],
                            gt[:, oA * F:(oA + DA[b]) * F].rearrange(
                                "p (d f) -> p f d", f=F
                            ),
                            axis=X,
                        )
                        aggB = wpool.tile([P, F], f32, tag="aggB")
                        rB = nc.vector.reduce_sum(
                            aggB[:],
                            gt[:, oB * F:(oB + DB[b]) * F].rearrange(
                                "p (d f) -> p f d", f=F
                            ),
                            axis=X,
                        )
                        rA._wait_ge(*wait_a[0])
                        rB._wait_ge(*wait_b[0])
                        agg = wpool.tile([P, F], f32, tag="agg")
                        nc.vector.tensor_add(agg[:], aggA[:], aggB[:])
                        aggT_ps = ppool.tile([F, P], f32, tag="tps")
                        nc.tensor.transpose(aggT_ps[:], agg[:], ident[:])
                        aggT = wpool.tile([F, P], f32, tag="aggT")
                        nc.scalar.activation(aggT[:], aggT_ps[:], Act.Copy)
                        mm_ps = ppool.tile([P, G], f32, tag="mmps")
                        nc.tensor.matmul(
                            mm_ps[:], lhsT=aggT[:], rhs=rhs_s[r][:],
                            start=True, stop=True,
                        )
                        root = root1 if r <= 2 else root2
                        t_sb = wpool.tile([P, G], f32, tag="tsb")
                        nc.vector.scalar_tensor_tensor(
                            t_sb[:], mm_ps[:], dcol, root[:, b, :],
                            op0=Alu.mult, op1=Alu.add,
                        )
                        if r == 1:
                            yo = wpool.tile([P, G1], bf16, tag="yout16")
                            nc.scalar.activation(yo[:], t_sb[:], Act.Relu, scale=dcol)
                            nc.sync.dma_start(
                                out=ag_in[2][b * P:(b + 1) * P, :], in_=yo[:]
                            )
                        elif r == 2:
                            out1 = wpool.tile([P, G1], f32, tag="out1")
                            nc.scalar.activation(out1[:], t_sb[:], Act.Relu)
                            hsum = wpool.tile([P, HID_C], f32, tag="hsum")
                            nc.vector.tensor_add(
                                hsum[:], out1[:, :HID_C], out1[:, HID_C:]
                            )
                            yo = wpool.tile([P, HID_C], f32, tag="yout")
                            nc.scalar.activation(
                                yo[:], hsum[:], Act.Copy, scale=dinvh[:, b:b + 1]
                            )
                            nc.sync.dma_start(
                                out=ag_in[3][b * P:(b + 1) * P, :], in_=yo[:]
                            )
                            hT_ps = ppool.tile([HID_C, P], f32, tag="tps")
                            nc.tensor.transpose(hT_ps[:], hsum[:], ident[:])
                            hT = wpool.tile([HID_C, P], f32, tag="aggT")
                            nc.scalar.activation(hT[:], hT_ps[:], Act.Copy)
                            r2_ps = ppool.tile([P, G2], f32, tag="mmps")
                            nc.tensor.matmul(
                                r2_ps[:], lhsT=hT[:], rhs=rootw2_s[:],
                                start=True, stop=True,
                            )
                            nc.vector.tensor_add(root2[:, b, :], r2_ps[:], b2rep[:])
                        elif r == 3:
                            yo = wpool.tile([P, G2], f32, tag="yout")
                            nc.scalar.activation(yo[:], t_sb[:], Act.Relu, scale=dcol)
                            nc.sync.dma_start(
                                out=ag_in[4][b * P:(b + 1) * P, :], in_=yo[:]
                            )
                        else:
                            ofin = wpool.tile([P, G2], f32, tag="out1")
                            nc.scalar.activation(ofin[:], t_sb[:], Act.Relu)
                            msum = wpool.tile([P, OUT_C], f32, tag="hsum")
                            nc.vector.tensor_add(
                                msum[:], ofin[:, :OUT_C], ofin[:, OUT_C:]
                            )
                            yo = wpool.tile([P, OUT_C], f32, tag="yout")
                            nc.scalar.activation(yo[:], msum[:], Act.Copy, scale=0.5)
                            nc.sync.dma_start(
                                out=out_d[b * P:(b + 1) * P, :], in_=yo[:]
                            )
                if r < 4 and r < max_round and not os.environ.get("GNN_SKIP_AG"):
                    nc.gpsimd.collective_compute(
                        "AllGather", Alu.bypass, replica_groups=rg,
                        ins=[ag_in[r + 1].ap().opt()], outs=[y[r + 1].ap().opt()],
                    )

    nc.compile()
    return nc


def build_in_maps(inputs: dict, cfg: Cfg, st: Struct) -> list:
    x = np.asarray(inputs["x"], dtype=np.float32)
    wmap = build_weight_inputs(inputs, cfg)
    in_maps = []
    for c in range(cfg.CORES):
        xs = np.zeros((cfg.NPC, cfg.IN_C), np.float32)
        mine = np.nonzero(st.pid // cfg.NPC == c)[0]
        loc = st.pid[mine] % cfg.NPC
        xs[loc] = x[mine]
        m = {
            "xs": xs,
            "idx16": np.ascontiguousarray(st.idx16[c]),
            "idx32": np.ascontiguousarray(st.idx32[c]),
        }
        m.update(wmap)
        in_maps.append(m)
    return in_maps


def assemble_output(results: list, cfg: Cfg, st: Struct) -> np.ndarray:
    full = np.concatenate(
        [np.asarray(results[c]["out"]) for c in range(cfg.CORES)], axis=0
    )
    return np.ascontiguousarray(full[st.pid]).astype(np.float32)


def kernel(**inputs) -> np.ndarray:
    from concourse.bass_utils import run_bass_kernel_spmd

    cfg = Cfg()
    st = build_structure(np.asarray(inputs["edge_index"]), cfg)
    nc = build_nc(cfg, st)
    in_maps = build_in_maps(inputs, cfg, st)
    res = run_bass_kernel_spmd(nc, in_maps, core_ids=list(range(cfg.CORES)))
    return assemble_output(res.results, cfg, st)


if __name__ == "__main__":
    pass


# revision 9
# speedup vs baseline: 11.3566x; 1.0262x over previous
"""Trainium2 Bass kernel for nn_BiARMA (2-layer ARMA GNN, K=2 stacks, T=2).

Math: A = D^-1/2 C D^-1/2 (C = edge-count matrix, deg by dst).
Key identity: norm[e] = dinv[src]*dinv[dst] factors, so
  segment_sum(out[src]*norm, dst) = dinv_dst * gather_sum(dinv_src*out[src])
-> every message-passing round is a pure row-gather-accumulate of a
pre-scaled node tensor.  Weights commute with aggregation, so matmuls
run on the aggregated tensor.

Distribution: dst-nodes sharded over 8 cores (graph parallel).  Each
core keeps a full replica of the current pre-scaled node tensor in its
DRAM, gathers rows for its local edges with the gpsimd dma_gather
ucode op, reduces padded per-node slots on DVE, applies weights on PE,
and AllGathers its updated shard each round.

Gather indices are int16 (<=32768 addressable rows), but the replica
has 50176 rows.  Two gather calls per group with OVERLAPPING windows:
call A reads rows [0, 32768) and call B reads rows [17408, 50176) of
the same replica.  Sources on cores 0-2 must use call A, cores 5-7
call B, and cores 3-4 (filled with the highest out-degree nodes) can
use either -- those flexible edges are assigned per destination to
balance the two calls, which makes the per-block padded slot counts
(max over cores x 128 partition rows) nearly tight: ~899 padded slot
columns vs 782 ideal vs 1387 for a disjoint-half split.
"""

import os
import sys
from dataclasses import dataclass, field

import numpy as np

sys.path.insert(0, "/opt/trn_rl_repo")

P = 128
WIN = 32768  # rows addressable by one int16-indexed gather call


@dataclass
class Cfg:
    N: int = 50000
    E: int = 800000
    IN_C: int = 64
    HID_C: int = 64
    OUT_C: int = 32
    K: int = 2
    CORES: int = 8
    # gather-tile budget, elements per partition per group (per dtype)
    group_budget_elems: int = 6144

    @property
    def blocks(self):
        return (self.N // self.CORES + 1 + P - 1) // P

    @property
    def NPC(self):
        return self.blocks * P

    @property
    def NREP(self):
        return self.CORES * self.NPC

    @property
    def OFFB(self):  # window-B base row
        return self.NREP - WIN


@dataclass
class Struct:
    DA: list
    DB: list
    col_off: list       # per-block column offset (A+B combined)
    a_off: list         # per-block offset into the A column space
    b_off: list         # per-block offset into the B column space
    tot_cols: int
    idx16: np.ndarray   # [CORES, 128, (TA+TB)*8] int16 wrapped+replicated
    idx32: np.ndarray   # [CORES, P, tot_cols] int32, -1 padded (deg helper)
    pid: np.ndarray
    a_cum: list = None
    b_cum: list = None
    TA: int = 0
    TB: int = 0
    groups: dict = field(default_factory=dict)


def build_structure(edge_index: np.ndarray, cfg: Cfg) -> Struct:
    src = np.asarray(edge_index[0], dtype=np.int64)
    dst = np.asarray(edge_index[1], dtype=np.int64)
    N, CORES, NPC, NB = cfg.N, cfg.CORES, cfg.NPC, cfg.blocks
    OFFB = cfg.OFFB

    # ---- core assignment: highest out-degree nodes fill cores 3,4 (the
    # cores whose pid range lies inside BOTH gather windows), everything
    # else round-robins over the remaining cores ----
    outdeg = np.bincount(src, minlength=N)
    o = np.argsort(-outdeg, kind="stable")
    core_of = np.empty(N, np.int64)
    nflex = 2 * NPC
    core_of[o[:nflex]] = np.where(np.arange(nflex) % 2 == 0, 3, 4)
    rest_cores = np.array([0, 1, 2, 5, 6, 7])
    core_of[o[nflex:]] = rest_cores[np.arange(N - nflex) % 6]

    # ---- call assignment per edge: by source core, flexible edges
    # (src on cores 3,4) greedily balance each destination's two calls ----
    sc = core_of[src]
    a_fixed = sc <= 2
    b_fixed = sc >= 5
    cA = np.zeros(N, np.int64)
    cB = np.zeros(N, np.int64)
    np.add.at(cA, dst[a_fixed], 1)
    np.add.at(cB, dst[b_fixed], 1)
    half = np.where(a_fixed, 0, 1).astype(np.int64)
    fidx = np.nonzero(~a_fixed & ~b_fixed)[0]
    fidx = fidx[np.argsort(dst[fidx], kind="stable")]
    fd = dst[fidx]
    # per-dst greedy balance, vectorized: within each dst's flexible run,
    # first |imb| edges go to the lighter side, then alternate
    runs = np.concatenate([[0], np.cumsum(np.bincount(fd, minlength=N))])
    pos = np.arange(fidx.shape[0]) - runs[fd]
    imb = (cA - cB)[fd]  # >0: A heavier -> first flex edges go to B
    rem = pos - np.abs(imb)
    go_b = np.where(rem < 0, imb > 0, (rem % 2 == 1) ^ (imb < 0))
    half[fidx] = go_b.astype(np.int64)
    cA = np.zeros(N, np.int64)
    cB = np.zeros(N, np.int64)
    np.add.at(cA, dst[half == 0], 1)
    np.add.at(cB, dst[half == 1], 1)

    # ---- local ordering within each core: lexicographic (cA, cB) desc
    # -> tight per-block padded widths ----
    local_of = np.empty(N, np.int64)
    for c in range(CORES):
        nodes = np.nonzero(core_of == c)[0]
        o2 = np.lexsort((-cB[nodes], -cA[nodes]))
        local_of[nodes[o2]] = np.arange(len(nodes))
    pid = core_of * NPC + local_of

    # window sanity: A-call sources have pid < WIN, B-call >= OFFB
    spid = pid[src]
    assert spid[half == 0].max() < WIN
    assert spid[half == 1].min() >= OFFB

    ecore = core_of[dst]
    dloc = local_of[dst]
    sloc = np.where(half == 0, spid, spid - OFFB)  # window-local index

    # per (call, core, node) counts -> per-block padded A/B widths
    cnt = np.zeros((2, CORES, NPC), np.int64)
    for h in (0, 1):
        for c in range(CORES):
            m = (ecore == c) & (half == h)
            cnt[h, c] = np.bincount(dloc[m], minlength=NPC)
    DA = cnt[0].reshape(CORES, NB, P).max(axis=(0, 2))
    DB = cnt[1].reshape(CORES, NB, P).max(axis=(0, 2))
    DA = np.maximum(DA, 1).tolist()
    DB = np.maximum(DB, 1).tolist()
    D = [DA[b] + DB[b] for b in range(NB)]
    col_off = np.concatenate([[0], np.cumsum(D)]).tolist()
    a_off = [col_off[b] for b in range(NB)]          # A slots first per block
    b_off = [col_off[b] + DA[b] for b in range(NB)]  # then B slots
    tot_cols = int(col_off[-1])

    # per-slot values, node-major layout [P, tot_cols]
    vals = np.full((CORES, P, tot_cols), -1, np.int64)
    eo = np.lexsort((dloc, ecore))
    ecore_s, dloc_s, sloc_s, half_s = ecore[eo], dloc[eo], sloc[eo], half[eo]
    aoff = np.asarray(a_off)
    boff = np.asarray(b_off)
    for c in range(CORES):
        m = ecore_s == c
        dl, sl, hh = dloc_s[m], sloc_s[m], half_s[m]
        for h in (0, 1):
            mh = hh == h
            dlh, slh = dl[mh], sl[mh]
            cth = np.bincount(dlh, minlength=NPC)
            starts = np.concatenate([[0], np.cumsum(cth)])[:-1]
            p_in = np.arange(dlh.shape[0]) - starts[dlh]
            b = dlh // P
            p = dlh % P
            col = (aoff if h == 0 else boff)[b] + p_in
            vals[c, p, col] = slh

    # int32 deg helper (-1 = pad)
    idx32 = vals.astype(np.int32)

    st = Struct(DA=DA, DB=DB, col_off=col_off, a_off=a_off, b_off=b_off,
                tot_cols=tot_cols, idx16=None, idx32=idx32, pid=pid)

    def make_groups(budget_cols):
        budget = max(budget_cols, max(D))
        groups = []
        b0 = 0
        while b0 < NB:
            b1 = b0
            tot = 0
            while b1 < NB and (tot + D[b1] <= budget or b1 == b0):
                tot += D[b1]
                b1 += 1
            groups.append((b0, b1))
            b0 = b1
        return groups

    be = cfg.group_budget_elems
    st.groups = {
        1: make_groups(be // cfg.IN_C),
        2: make_groups(be // (cfg.K * cfg.HID_C)),
        3: make_groups(be // cfg.HID_C),
        4: make_groups(be // (cfg.K * cfg.OUT_C)),
    }

    # int16 gather arrays.  For ANY contiguous block range, call A reads
    # the A-columns of blocks b0..b1 in block order, call B the
    # B-columns.  Store two wrapped arrays (all A-columns block-major,
    # then all B-columns) so every call's index slice is contiguous.
    # Flat call position i -> (partition i%128, out column i//128);
    # wrapped storage (i%16, i//16), replicated x8 to 128 partitions.
    a_cum = np.concatenate([[0], np.cumsum(DA)]).astype(np.int64)
    b_cum = np.concatenate([[0], np.cumsum(DB)]).astype(np.int64)
    TA, TB = int(a_cum[-1]), int(b_cum[-1])
    st.a_cum = a_cum.tolist()
    st.b_cum = b_cum.tolist()
    st.TA, st.TB = TA, TB

    # pad rows: phantom (always-zero) rows inside each window
    PADA = NPC - 1               # core 0, top local -- phantom
    PADB = CORES * NPC - 1 - OFFB  # core 7, top local, window-B-relative
    assert PADA < WIN and 0 <= PADB < WIN

    idx16 = np.empty((CORES, 16, (TA + TB) * 8), np.int16)
    for c in range(CORES):
        va = np.empty((P, TA), np.int64)
        vb = np.empty((P, TB), np.int64)
        for b in range(NB):
            va[:, a_cum[b]:a_cum[b + 1]] = \
                vals[c][:, a_off[b]:a_off[b] + DA[b]]
            vb[:, b_cum[b]:b_cum[b + 1]] = \
                vals[c][:, b_off[b]:b_off[b] + DB[b]]
        va = np.where(va < 0, PADA, va)
        vb = np.where(vb < 0, PADB, vb)
        both = np.concatenate([va, vb], axis=1).astype(np.int16)
        # column c', partition p -> flat i = c'*128 + p -> (i%16, i//16):
        # wrapped[r, c'*8 + q] with p = q*16 + r
        w = both.reshape(16, 8, TA + TB, order="F")
        w2 = np.transpose(w, (0, 2, 1)).reshape(16, (TA + TB) * 8)
        idx16[c] = w2
    st.idx16 = np.tile(idx16, (1, 8, 1))
    return st


# packed-weight column layout in the single [128, 768] f32 input:
# w1bd 0:128 | rootw1 128:256 | initw1 256:384 | rootw2 384:448 |
# initw2 448:512 | w2bd 512:576 | b1 576:704 (row 0) | b2 704:768 (row 0)
WPACK_COLS = 768


def build_weight_inputs(inp: dict, cfg: Cfg) -> dict:
    K, IN_C, HID_C, OUT_C = cfg.K, cfg.IN_C, cfg.HID_C, cfg.OUT_C

    rootw1 = np.transpose(inp["root_w1"][0], (1, 0, 2)).reshape(IN_C, K * HID_C)
    b1row = inp["b1"][0, :, 0, :].reshape(1, K * HID_C)
    initw1 = np.transpose(inp["init_w1"], (1, 0, 2)).reshape(IN_C, K * HID_C)
    w1bd = np.zeros((K * HID_C, K * HID_C), np.float32)
    for k in range(K):
        w1bd[k * HID_C:(k + 1) * HID_C, k * HID_C:(k + 1) * HID_C] = inp["w1"][0, k]

    # 0.5 absorbed: round-2 h-stage feeds the UNhalved stack sum into root2
    rootw2 = 0.5 * np.transpose(inp["root_w2"][0], (1, 0, 2)).reshape(HID_C, K * OUT_C)
    b2row = inp["b2"][0, :, 0, :].reshape(1, K * OUT_C)
    initw2 = np.transpose(inp["init_w2"], (1, 0, 2)).reshape(HID_C, K * OUT_C)
    w2bd = np.zeros((K * OUT_C, K * OUT_C), np.float32)
    for k in range(K):
        w2bd[k * OUT_C:(k + 1) * OUT_C, k * OUT_C:(k + 1) * OUT_C] = inp["w2"][0, k]

    wp = np.zeros((128, WPACK_COLS), np.float32)
    wp[:, 0:128] = w1bd
    wp[:IN_C, 128:256] = rootw1
    wp[:IN_C, 256:384] = initw1
    wp[:HID_C, 384:448] = rootw2
    wp[:HID_C, 448:512] = initw2
    wp[:K * OUT_C, 512:576] = w2bd
    wp[0, 576:704] = b1row[0]
    wp[0, 704:768] = b2row[0]
    return {"wpack": wp}


def build_nc(cfg: Cfg, st: Struct):
    import concourse.bacc as bacc
    import concourse.bass as bass
    import concourse.mybir as mybir
    import concourse.tile as tile
    from concourse.masks import make_identity

    f32 = mybir.dt.float32
    bf16 = mybir.dt.bfloat16
    i16 = mybir.dt.int16
    i32 = mybir.dt.int32
    X = mybir.AxisListType.X
    Alu = mybir.AluOpType
    Act = mybir.ActivationFunctionType

    K, IN_C, HID_C, OUT_C = cfg.K, cfg.IN_C, cfg.HID_C, cfg.OUT_C
    G1 = K * HID_C   # 128
    G2 = K * OUT_C   # 64
    NB = cfg.blocks
    NPC, NREP, OFFB = cfg.NPC, cfg.NREP, cfg.OFFB
    DA, DB = st.DA, st.DB
    TA, TB = st.TA, st.TB
    a_cum, b_cum = st.a_cum, st.b_cum
    WTOT = (TA + TB) * 8

    nc = bacc.Bacc(
        "TRN2",
        target_bir_lowering=False,
        debug=False,
        num_devices=cfg.CORES,
    )

    # ---- kernel I/O ----
    xs = nc.dram_tensor("xs", [NPC, IN_C], f32, kind="ExternalInput")
    idx16_d = nc.dram_tensor("idx16", [P, WTOT], i16, kind="ExternalInput")
    dinv_d = nc.dram_tensor("dinv", [P, NB], f32, kind="ExternalInput")
    wpack_d = nc.dram_tensor("wpack", [P, WPACK_COLS], f32, kind="ExternalInput")
    out_d = nc.dram_tensor("out", [NPC, OUT_C], f32, kind="ExternalOutput")

    # ---- internal DRAM (y2 in bf16: halves AllGather + gather bytes,
    # 128 bf16 = 256B descriptors still satisfy the gather constraint) ----
    y = {
        1: nc.dram_tensor("y1", [NREP, IN_C], f32, addr_space="Shared"),
        2: nc.dram_tensor("y2", [NREP, G1], bf16, addr_space="Shared"),
        3: nc.dram_tensor("y3", [NREP, HID_C], f32, addr_space="Shared"),
        4: nc.dram_tensor("y4", [NREP, G2], f32, addr_space="Shared"),
    }
    ag_in = {
        1: nc.dram_tensor("agin1", [NPC, IN_C], f32),
        2: nc.dram_tensor("agin2", [NPC, G1], bf16),
        3: nc.dram_tensor("agin3", [NPC, HID_C], f32),
        4: nc.dram_tensor("agin4", [NPC, G2], f32),
    }
    FW = {1: IN_C, 2: G1, 3: HID_C, 4: G2}
    GW = {1: G1, 2: G1, 3: G2, 4: G2}
    YDT = {1: f32, 2: bf16, 3: f32, 4: f32}

    rg = [list(range(cfg.CORES))]

    max_gt = {
        dt: max(
            (max((st.col_off[b1] - st.col_off[b0]) * FW[r]
                 for (b0, b1) in st.groups[r])
             for r in (1, 2, 3, 4) if YDT[r] == dt),
            default=0,
        )
        for dt in (f32, bf16)
    }

    NSEM = 8
    dsems = [nc.alloc_semaphore(f"gsem{i}") for i in range(NSEM)]
    sem_count = [0] * NSEM
    gidx = [0]  # global gather-call counter
    wait_a = [None]
    wait_b = [None]

    with tile.TileContext(nc) as tc:
        with (
            tc.tile_pool(name="const", bufs=1) as cpool,
            tc.tile_pool(name="gather", bufs=3) as gpool,
            tc.tile_pool(name="work", bufs=3) as wpool,
            tc.tile_pool(name="psum", bufs=3, space="PSUM") as ppool,
        ):
            # ---------- constants ----------
            ident = cpool.tile([P, P], f32)
            make_identity(nc, ident[:])

            wsb = cpool.tile([P, WPACK_COLS], f32)
            nc.sync.dma_start(out=wsb[:], in_=wpack_d[:, :])
            w1bd_s = wsb[:, 0:128]
            rootw1_s = wsb[0:IN_C, 128:256]
            initw1_s = wsb[0:IN_C, 256:384]
            rootw2_s = wsb[0:HID_C, 384:448]
            initw2_s = wsb[0:HID_C, 448:512]
            w2bd_s = wsb[0:G2, 512:576]
            b1_s = wsb[0:1, 576:704]
            b2_s = wsb[0:1, 704:768]
            rhs_s = {1: initw1_s, 2: w1bd_s, 3: initw2_s, 4: w2bd_s}

            ones1 = cpool.tile([1, P], f32)
            nc.vector.memset(ones1[:], 1.0)
            b1rep = cpool.tile([P, G1], f32)
            b2rep = cpool.tile([P, G2], f32)
            bps = ppool.tile([P, G1], f32, tag="mmps")
            nc.tensor.matmul(bps[:], lhsT=ones1[:], rhs=b1_s, start=True, stop=True)
            nc.vector.tensor_copy(b1rep[:], bps[:])
            bps2 = ppool.tile([P, G2], f32, tag="mmps")
            nc.tensor.matmul(bps2[:], lhsT=ones1[:], rhs=b2_s, start=True, stop=True)
            nc.vector.tensor_copy(b2rep[:], bps2[:])

            # ---------- gather indices + degrees ----------
            idx16_s = cpool.tile([P, WTOT], i16)
            nc.sync.dma_start(out=idx16_s[:], in_=idx16_d[:, :])

            root1 = cpool.tile([P, NB, G1], f32)
            root2 = cpool.tile([P, NB, G2], f32)
            dinv = cpool.tile([P, NB], f32)
            dinvh = cpool.tile([P, NB], f32)
            nc.sync.dma_start(out=dinv[:], in_=dinv_d[:, :])
            nc.vector.tensor_scalar_mul(dinvh[:], dinv[:], 0.5)
            with tc.tile_pool(name="prolog", bufs=1) as qpool:
                # ---------- roots + Y1 ----------
                x_s = qpool.tile([P, NB, IN_C], f32)
                for b in range(NB):
                    nc.sync.dma_start(
                        out=x_s[:, b, :], in_=xs[b * P:(b + 1) * P, :]
                    )
                for b in range(NB):
                    dcol = dinv[:, b:b + 1]
                    xT_ps = ppool.tile([IN_C, P], f32, tag="tps")
                    nc.tensor.transpose(xT_ps[:], x_s[:, b, :], ident[:])
                    xT = wpool.tile([IN_C, P], f32, tag="aggT")
                    nc.scalar.activation(xT[:], xT_ps[:], Act.Copy)
                    r1_ps = ppool.tile([P, G1], f32, tag="mmps")
                    nc.tensor.matmul(
                        r1_ps[:], lhsT=xT[:], rhs=rootw1_s, start=True, stop=True
                    )
                    nc.vector.tensor_add(root1[:, b, :], r1_ps[:], b1rep[:])
                    y1b = wpool.tile([P, IN_C], f32, tag="yout")
                    nc.scalar.activation(y1b[:], x_s[:, b, :], Act.Copy, scale=dcol)
                    nc.sync.dma_start(
                        out=ag_in[1][b * P:(b + 1) * P, :], in_=y1b[:]
                    )
            if not os.environ.get("GNN_SKIP_AG0"):
                nc.gpsimd.collective_compute(
                    "AllGather", Alu.bypass, replica_groups=rg,
                    ins=[ag_in[1].ap().opt()], outs=[y[1].ap().opt()],
                )

            # ---------- 4 message-passing rounds ----------
            max_round = int(os.environ.get("GNN_STAGE", "4"))
            reps = int(os.environ.get("GNN_REPS", "1"))

            def gather_group(r, b0, b1):
                """Issue the A and B dma_gather calls for blocks [b0,b1);
                returns the gather tile (cols: A of b0..b1, then B)."""
                F = FW[r]
                dt = YDT[r]
                nA = (a_cum[b1] - a_cum[b0]) * P
                nB = (b_cum[b1] - b_cum[b0]) * P
                ncols = (nA + nB) // P
                gt = gpool.tile(
                    [P, max_gt[dt]], dt,
                    tag="gt32" if dt == f32 else "gt16",
                )
                outA = gt[:, :nA // P * F].rearrange("p (c f) -> p c f", f=F)
                outB = gt[:, nA // P * F:ncols * F].rearrange(
                    "p (c f) -> p c f", f=F
                )
                ixA = idx16_s[:, a_cum[b0] * 8:a_cum[b1] * 8]
                ixB = idx16_s[:, (TA + b_cum[b0]) * 8:(TA + b_cum[b1]) * 8]
                yv = y[r]
                sA = (2 * gidx[0]) % NSEM
                sB = (2 * gidx[0] + 1) % NSEM
                gidx[0] += 1
                sem_count[sA] += 16
                sem_count[sB] += 16
                nc.gpsimd.dma_gather(
                    out_ap=outA, in_ap=yv[0:WIN, :], idxs_ap=ixA,
                    num_idxs=nA, num_idxs_reg=nA, elem_size=F,
                    single_packet=False,
                ).then_inc(dsems[sA], 16)
                nc.gpsimd.dma_gather(
                    out_ap=outB, in_ap=yv[OFFB:NREP, :], idxs_ap=ixB,
                    num_idxs=nB, num_idxs_reg=nB, elem_size=F,
                    single_packet=False,
                ).then_inc(dsems[sB], 16)
                wait_a[0] = (dsems[sA], sem_count[sA])
                wait_b[0] = (dsems[sB], sem_count[sB])
                return gt, nA // P

            for rep in range(reps):
              for r in (1, 2, 3, 4):
                if r > max_round:
                    break
                F = FW[r]
                G = GW[r]
                for (b0, b1) in st.groups[r]:
                    gt, colsA = gather_group(r, b0, b1)
                    for b in range(b0, b1):
                        dcol = dinv[:, b:b + 1]
                        oA = a_cum[b] - a_cum[b0]
                        oB = colsA + (b_cum[b] - b_cum[b0])
                        aggA = wpool.tile([P, F], f32, tag="aggA")
                        rA = nc.vector.reduce_sum(
                            aggA[:],
                            gt[:, oA * F:(oA + DA[b]) * F].rearrange(
                                "p (d f) -> p f d", f=F
                            ),
                            axis=X,
                        )
                        aggB = wpool.tile([P, F], f32, tag="aggB")
                        rB = nc.vector.reduce_sum(
                            aggB[:],
                            gt[:, oB * F:(oB + DB[b]) * F].rearrange(
                                "p (d f) -> p f d", f=F
                            ),
                            axis=X,
                        )
                        rA._wait_ge(*wait_a[0])
                        rB._wait_ge(*wait_b[0])
                        agg = wpool.tile([P, F], f32, tag="agg")
                        nc.vector.tensor_add(agg[:], aggA[:], aggB[:])
                        aggT_ps = ppool.tile([F, P], f32, tag="tps")
                        nc.tensor.transpose(aggT_ps[:], agg[:], ident[:])
                        aggT = wpool.tile([F, P], f32, tag="aggT")
                        nc.scalar.activation(aggT[:], aggT_ps[:], Act.Copy)
                        mm_ps = ppool.tile([P, G], f32, tag="mmps")
                        nc.tensor.matmul(
                            mm_ps[:], lhsT=aggT[:], rhs=rhs_s[r],
                            start=True, stop=True,
                        )
                        root = root1 if r <= 2 else root2
                        t_sb = wpool.tile([P, G], f32, tag="tsb")
                        nc.vector.scalar_tensor_tensor(
                            t_sb[:], mm_ps[:], dcol, root[:, b, :],
                            op0=Alu.mult, op1=Alu.add,
                        )
                        if r == 1:
                            yo = wpool.tile([P, G1], bf16, tag="yout16")
                            nc.scalar.activation(yo[:], t_sb[:], Act.Relu, scale=dcol)
                            nc.sync.dma_start(
                                out=ag_in[2][b * P:(b + 1) * P, :], in_=yo[:]
                            )
                        elif r == 2:
                            out1 = wpool.tile([P, G1], f32, tag="out1")
                            nc.scalar.activation(out1[:], t_sb[:], Act.Relu)
                            hsum = wpool.tile([P, HID_C], f32, tag="hsum")
                            nc.vector.tensor_add(
                                hsum[:], out1[:, :HID_C], out1[:, HID_C:]
                            )
                            yo = wpool.tile([P, HID_C], f32, tag="yout")
                            nc.scalar.activation(
                                yo[:], hsum[:], Act.Copy, scale=dinvh[:, b:b + 1]
                            )
                            nc.sync.dma_start(
                                out=ag_in[3][b * P:(b + 1) * P, :], in_=yo[:]
                            )
                            hT_ps = ppool.tile([HID_C, P], f32, tag="tps")
                            nc.tensor.transpose(hT_ps[:], hsum[:], ident[:])
                            hT = wpool.tile([HID_C, P], f32, tag="aggT")
                            nc.scalar.activation(hT[:], hT_ps[:], Act.Copy)
                            r2_ps = ppool.tile([P, G2], f32, tag="mmps")
                            nc.tensor.matmul(
                                r2_ps[:], lhsT=hT[:], rhs=rootw2_s,
                                start=True, stop=True,
                            )
                            nc.vector.tensor_add(root2[:, b, :], r2_ps[:], b2rep[:])
                        elif r == 3:
                            yo = wpool.tile([P, G2], f32, tag="yout")
                            nc.scalar.activation(yo[:], t_sb[:], Act.Relu, scale=dcol)
                            nc.sync.dma_start(
                                out=ag_in[4][b * P:(b + 1) * P, :], in_=yo[:]
                            )
                        else:
                            ofin = wpool.tile([P, G2], f32, tag="out1")
                            nc.scalar.activation(ofin[:], t_sb[:], Act.Relu)
                            msum = wpool.tile([P, OUT_C], f32, tag="hsum")
                            nc.vector.tensor_add(
                                msum[:], ofin[:, :OUT_C], ofin[:, OUT_C:]
                            )
                            yo = wpool.tile([P, OUT_C], f32, tag="yout")
                            nc.scalar.activation(yo[:], msum[:], Act.Copy, scale=0.5)
                            nc.sync.dma_start(
                                out=out_d[b * P:(b + 1) * P, :], in_=yo[:]
                            )
                if r < 4 and r < max_round and not os.environ.get("GNN_SKIP_AG"):
                    nc.gpsimd.collective_compute(
                        "AllGather", Alu.bypass, replica_groups=rg,
                        ins=[ag_in[r + 1].ap().opt()], outs=[y[r + 1].ap().opt()],
                    )

    nc.compile()
    return nc


def build_in_maps(inputs: dict, cfg: Cfg, st: Struct) -> list:
    x = np.asarray(inputs["x"], dtype=np.float32)
    wmap = build_weight_inputs(inputs, cfg)
    # host-side dinv: deg by dst, dinv = deg**-0.5 (0 where deg==0)
    dst = np.asarray(inputs["edge_index"][1], dtype=np.int64)
    deg = np.bincount(dst, minlength=cfg.N).astype(np.float64)
    dinv_n = np.where(deg > 0, deg ** -0.5, 0.0).astype(np.float32)
    in_maps = []
    for c in range(cfg.CORES):
        xs = np.zeros((cfg.NPC, cfg.IN_C), np.float32)
        dv = np.zeros(cfg.NPC, np.float32)
        mine = np.nonzero(st.pid // cfg.NPC == c)[0]
        loc = st.pid[mine] % cfg.NPC
        xs[loc] = x[mine]
        dv[loc] = dinv_n[mine]
        m = {
            "xs": xs,
            "idx16": np.ascontiguousarray(st.idx16[c]),
            # dinv[p, b] = dinv of local node b*128+p
            "dinv": np.ascontiguousarray(
                dv.reshape(cfg.blocks, P).T
            ),
        }
        m.update(wmap)
        in_maps.append(m)
    return in_maps


def assemble_output(results: list, cfg: Cfg, st: Struct) -> np.ndarray:
    full = np.concatenate(
        [np.asarray(results[c]["out"]) for c in range(cfg.CORES)], axis=0
    )
    return np.ascontiguousarray(full[st.pid]).astype(np.float32)


def kernel(**inputs) -> np.ndarray:
    from concourse.bass_utils import run_bass_kernel_spmd

    cfg = Cfg()
    st = build_structure(np.asarray(inputs["edge_index"]), cfg)
    nc = build_nc(cfg, st)
    in_maps = build_in_maps(inputs, cfg, st)
    res = run_bass_kernel_spmd(nc, in_maps, core_ids=list(range(cfg.CORES)))
    return assemble_output(res.results, cfg, st)


if __name__ == "__main__":
    pass
